# revision 1
# baseline (speedup 1.0000x reference)
"""Trainium2 Bass kernel: Mistral-style GQA attention with sliding-window mask.

Problem: hidden [1,2048,4096] -> Wq/Wk/Wv projections (32 q heads, 8 kv heads,
head_dim 128) -> RoPE -> sliding-window (1024) causal attention -> Wo.

Sharding: tensor-parallel over heads on 8 NeuronCores. Core i owns KV head i
and query heads 4i..4i+3 (Wq/Wk/Wv row-sharded, Wo column-sharded). Each core
computes partial_i = attn_heads_i @ Wo_i^T in HBM; host sums the 8 partials
(the TP all-reduce) to produce the full output.

On-device per core:
  phase A: stream H^T tiles from HBM, cast bf16, matmul into Q^T/K^T/V^T
           (layout [head_dim, seq]), fused RoPE on Q/K out of PSUM,
           V^T transposed to natural [seq, head_dim] via DMA-transpose.
  phase B: block-sparse attention per 512-query chunk: scores^T = K^T-block
           x Q^T-chunk on PE, exp on ACT (PSUM->SBUF bf16), static triangular
           masks on DVE, P@V and row-sum denominators on PE (ones-vector
           matmul), reciprocal+partition_broadcast for normalization, then
           Wo matmuls and fp32 output copy.
"""

import sys

for _p in ("/opt/trn_rl_repo", "/root/.axon_site/_ro/trn_rl_repo"):
    if _p not in sys.path:
        sys.path.insert(0, _p)

import numpy as np
import ml_dtypes

import concourse.bass as bass  # noqa: F401  (registers engine classes)
import concourse.mybir as mybir
import concourse.tile as tile
from concourse import bacc
from concourse.bass_utils import run_bass_kernel_spmd

S = 2048
HID = 4096
D = 128
NQH = 4          # query heads per core
NCORES = 8
SC = 512         # seq chunk
NCH = S // SC
KT = HID // 128  # contraction tiles
WINDOW = 1024
ROPE_BASE = 10000.0
SCALE = 1.0 / float(np.sqrt(D))

F32 = mybir.dt.float32
BF16 = mybir.dt.bfloat16
MULT = mybir.AluOpType.mult
ADD = mybir.AluOpType.add
SUB = mybir.AluOpType.subtract
EXP = mybir.ActivationFunctionType.Exp

# ptb slot layout: slot sl = kb - 4c + 8 for key-block kb in chunk c.
# exp-written region per slot, and statically-zero (memset once) regions.
def _slot_region(sl):
    lo = 128 * (sl - 8) if sl >= 8 else 0
    hi = 512 if sl >= 3 else 128 * (sl + 1)
    return lo, hi

_INVIS = []
for _sl in range(12):
    _lo, _hi = _slot_region(_sl)
    if _lo > 0:
        _INVIS.append((_sl, 0, _lo))
    if _hi < 512:
        _INVIS.append((_sl, _hi, 512))


def _program(tc, dr, out, niter=1, fused=True):
    nc = tc.nc
    ht, wqt, wkt, wvt, wot = dr["ht"], dr["wqt"], dr["wkt"], dr["wvt"], dr["wot"]
    ctab, stab, mcaus, mwin = dr["ctab"], dr["stab"], dr["mcaus"], dr["mwin"]

    def _copy(eng, out_ap, in_ap):
        if eng is nc.scalar:
            eng.copy(out_ap, in_ap)
        else:
            eng.tensor_copy(out_ap, in_ap)

    cast_engines = [nc.vector, nc.gpsimd]
    cast_idx = [0]

    def cast_rr(out_ap, in_ap):
        _copy(cast_engines[cast_idx[0] % 2], out_ap, in_ap)
        cast_idx[0] += 1

    # psum-reading copy engines (gpsimd cannot touch PSUM); ACT-heavy split
    pcopy_engines = [nc.scalar, nc.vector]
    pcopy_idx = [0]

    def pcopy_rr(out_ap, in_ap):
        _copy(pcopy_engines[pcopy_idx[0] % 2], out_ap, in_ap)
        pcopy_idx[0] += 1

    from contextlib import ExitStack
    for _it in range(niter):
        with ExitStack() as ctx:
            pw = ctx.enter_context(tc.tile_pool(name="persist", bufs=1))
            pst = ctx.enter_context(tc.tile_pool(name="stage", bufs=13))
            prt = ctx.enter_context(tc.tile_pool(name="ropet", bufs=2))

            wqb = pw.tile([128, KT * 512], BF16, name="wqb")
            wkb = pw.tile([128, KT * 128], BF16, name="wkb")
            wvb = pw.tile([128, KT * 128], BF16, name="wvb")
            wob = pw.tile([128, NQH * HID], BF16, name="wob")
            qtb = [pw.tile([128, S], BF16, name=f"qtb{h}") for h in range(NQH)]
            ktb = pw.tile([128, S], BF16, name="ktb")
            vtb = pw.tile([128, S], BF16, name="vtb")
            vnat = pw.tile([128, S], BF16, name="vnat")
            cs_t = pw.tile([128, S], F32, name="cs_t")
            sn_t = pw.tile([128, S], F32, name="sn_t")
            mc_t = pw.tile([128, 128], BF16, name="mc_t")
            mw_t = pw.tile([128, 128], BF16, name="mw_t")
            ones_t = pw.tile([128, 1], BF16, name="ones_t")

            # constants
            nc.sync.dma_start(cs_t[:], ctab[:])
            nc.sync.dma_start(sn_t[:], stab[:])
            nc.sync.dma_start(mc_t[:], mcaus[:])
            nc.sync.dma_start(mw_t[:], mwin[:])
            nc.gpsimd.memset(ones_t[:], 1.0)

            # rope helper state
            rope_state = [0]

            def _rope(dst, p, c):
                """dst[bf16 [128,512] slice] = rope(p [psum f32 [128,512]]) at chunk c.

                cs_t is cos duplicated across both halves; sn_t is sign-baked
                sin: rows 0:64 = -sin, rows 64:128 = +sin, so
                out = q*cos + rot(q)*sn with rot a plain half-swap.
                """
                use_gp = rope_state[0] % 5 >= 3
                rope_state[0] += 1
                csl = cs_t[:, SC * c:SC * (c + 1)]
                snl = sn_t[:, SC * c:SC * (c + 1)]
                if use_gp:
                    pre = prt.tile([128, 512], F32, tag="rpre", name="rpre", bufs=1)
                    rot = prt.tile([128, 512], F32, tag="rrot", name="rrot", bufs=1)
                    nc.scalar.copy(pre[:], p[:])
                    nc.scalar.copy(rot[0:64, :], p[64:128, :])
                    nc.scalar.copy(rot[64:128, :], p[0:64, :])
                    g1 = prt.tile([128, 512], F32, tag="rt1", name="g1")
                    g2 = prt.tile([128, 512], F32, tag="rt2", name="g2")
                    nc.gpsimd.tensor_tensor(g1[:], pre[:], csl, MULT)
                    nc.gpsimd.tensor_tensor(g2[:], rot[:], snl, MULT)
                    nc.gpsimd.tensor_tensor(dst[:, :], g1[:], g2[:], ADD)
                    return
                e = nc.vector
                t1 = prt.tile([64, 512], F32, tag="rt1", name="rt1")
                t2 = prt.tile([64, 512], F32, tag="rt2", name="rt2")
                e.tensor_tensor(t1[:], p[0:64, :], csl[0:64, :], MULT)
                e.tensor_tensor(t2[:], p[64:128, :], snl[0:64, :], MULT)
                e.tensor_tensor(dst[0:64, :], t1[:], t2[:], ADD)
                t3 = prt.tile([64, 512], F32, tag="rt1", name="rt3")
                t4 = prt.tile([64, 512], F32, tag="rt2", name="rt4")
                e.tensor_tensor(t3[:], p[64:128, :], csl[64:128, :], MULT)
                e.tensor_tensor(t4[:], p[0:64, :], snl[64:128, :], MULT)
                e.tensor_tensor(dst[64:128, :], t3[:], t4[:], ADD)

            phb = ctx.enter_context(tc.tile_pool(name="htbp", bufs=8))
            ppt = ctx.enter_context(tc.tile_pool(name="ptp", bufs=7))
            pmisc = ctx.enter_context(tc.tile_pool(name="miscb", bufs=2))
            pat = ctx.enter_context(tc.tile_pool(name="atbp", bufs=10))
            posb = ctx.enter_context(tc.tile_pool(name="osbp", bufs=4))

            # per-chunk pipeline: projections (+weight streaming on c==0),
            # then block-sparse attention, then Wo partial + output store.
            # fused=True interleaves the two stages per chunk; False runs all
            # projections first, then all attention chunks.
            def proj_stage(c):
                with tc.tile_pool(name="projps", bufs=6, space="PSUM") as ppp:
                    ps6 = [ppp.tile([128, 512], F32, tag="proj", name=f"proj{c}_{i}")
                           for i in range(6)]
                    for k in range(KT):
                        if c == 1:
                            # Wo weights are first needed when chunk 0's Wo is
                            # drained during attn(1); loading here keeps chunk 0
                            # (already DMA-heavy) lighter.
                            stwo = pst.tile([128, 512], F32, tag="stage", name="stwo")
                            nc.sync.dma_start(stwo[:], wot[128 * (k % 4):128 * (k % 4 + 1),
                                                           512 * (k // 4):512 * (k // 4 + 1)])
                            cast_rr(wob[:, HID * (k % 4) + 512 * (k // 4):
                                            HID * (k % 4) + 512 * (k // 4 + 1)], stwo[:])
                        if c == 0:
                            stw = pst.tile([128, 512], F32, tag="stage", name="stwq")
                            nc.sync.dma_start(stw[:], wqt[128 * k:128 * (k + 1), :])
                            cast_rr(wqb[:, 512 * k:512 * (k + 1)], stw[:])
                            stkv = pst.tile([128, 256], F32, tag="stage", name="stkv")
                            nc.sync.dma_start(stkv[:, 0:128], wkt[128 * k:128 * (k + 1), :])
                            nc.sync.dma_start(stkv[:, 128:256], wvt[128 * k:128 * (k + 1), :])
                            cast_rr(wkb[:, 128 * k:128 * (k + 1)], stkv[:, 0:128])
                            cast_rr(wvb[:, 128 * k:128 * (k + 1)], stkv[:, 128:256])
                        st = pst.tile([128, 512], F32, tag="stage", name="sth")
                        nc.sync.dma_start(st[:], ht[128 * k:128 * (k + 1), SC * c:SC * (c + 1)])
                        hb = phb.tile([128, 512], BF16, tag="htb", name="hb")
                        _copy(nc.scalar if k % 2 == 0 else nc.vector, hb[:], st[:])
                        first, last = k == 0, k == KT - 1
                        for h in range(NQH):
                            nc.tensor.matmul(ps6[h][:], wqb[:, 512 * k + 128 * h:512 * k + 128 * (h + 1)],
                                             hb[:], start=first, stop=last)
                        nc.tensor.matmul(ps6[4][:], wkb[:, 128 * k:128 * (k + 1)], hb[:],
                                         start=first, stop=last)
                        nc.tensor.matmul(ps6[5][:], wvb[:, 128 * k:128 * (k + 1)], hb[:],
                                         start=first, stop=last)
                    _rope(ktb[:, SC * c:SC * (c + 1)], ps6[4], c)
                    for h in range(NQH):
                        _rope(qtb[h][:, SC * c:SC * (c + 1)], ps6[h], c)
                    nc.scalar.copy(vtb[:, SC * c:SC * (c + 1)], ps6[5][:])
                    for b4 in range(4):
                        nc.sync.dma_start_transpose(
                            vnat[:, 128 * (4 * c + b4):128 * (4 * c + b4 + 1)],
                            vtb[:, SC * c + 128 * b4:SC * c + 128 * (b4 + 1)])

            def emit_wo_group(pop, wc, wj, wn, watbs):
                """One Wo output tile [128q, 512hid] for chunk wc: 4 head-MMs,
                PSUM->SBUF copy, store."""
                po = pop.tile([128, 512], F32, tag="po", name="po")
                for h in range(NQH):
                    nc.tensor.matmul(po[:], watbs[h][:, 128 * wj:128 * (wj + 1)],
                                     wob[:, HID * h + 512 * wn:HID * h + 512 * (wn + 1)],
                                     start=(h == 0), stop=(h == NQH - 1))
                ob = posb.tile([128, 512], F32, tag="osb", name="osb")
                pcopy_rr(ob[:], po[:])
                nc.sync.dma_start(out[SC * wc + 128 * wj:SC * wc + 128 * (wj + 1),
                                      512 * wn:512 * (wn + 1)], ob[:])

            def attn_stage(c, prev):
                # ---- attention for this chunk (past K/V only: sliding window),
                # with the PREVIOUS chunk's Wo matmul groups interleaved between
                # key-blocks so PE has independent work during exp waits.
                wo_pending = []
                if prev is not None:
                    pc, patbs = prev
                    wo_pending = [(pc, j, n, patbs) for j in range(4) for n in range(8)]
                with tc.tile_pool(name="scps", bufs=3, space="PSUM") as psc, \
                     tc.tile_pool(name="pvps", bufs=2, space="PSUM") as ppv, \
                     tc.tile_pool(name="denps", bufs=2, space="PSUM") as pdn, \
                     tc.tile_pool(name="outps", bufs=1, space="PSUM") as pop:
                    kbs = list(range(max(0, 4 * c - 8), 4 * c + 4))
                    first_kb, last_kb = kbs[0], kbs[-1]
                    # give PE independent work while DVE runs this chunk's rope
                    for _ in range(min(6, len(wo_pending))):
                        emit_wo_group(pop, *wo_pending.pop(0))
                    atbs = []
                    for h0 in range(0, NQH, 2):
                        # process a PAIR of heads per key-block sweep: two
                        # independent score/exp chains per step keep ACT fed.
                        pvs = [ppv.tile([128, 512], F32, tag="pv", name="pv")
                               for _ in range(2)]
                        dens = [pdn.tile([1, 512], F32, tag="den", name="den")
                                for _ in range(2)]

                        def emit_pv(kb, pts):
                            # accumulate P@V and row-sums over exact visible slices.
                            # start=True on the first key-block clears the bank's
                            # has_written bits; later blocks auto-overwrite elements
                            # they touch first and accumulate elsewhere.
                            sl = kb - 4 * c + 8
                            lo, hi = _slot_region(sl)
                            vsl = vnat[:, 128 * kb:128 * (kb + 1)]
                            for i in range(2):
                                nc.tensor.matmul(pvs[i][:, lo:hi], vsl, pts[i][:, lo:hi],
                                                 start=(kb == first_kb), stop=(kb == last_kb),
                                                 skip_group_check=True)
                                nc.tensor.matmul(dens[i][:, lo:hi], ones_t[:], pts[i][:, lo:hi],
                                                 start=(kb == first_kb), stop=(kb == last_kb),
                                                 skip_group_check=True)

                        prev = None
                        for kb in kbs:
                            sl = kb - 4 * c + 8
                            lo, hi = _slot_region(sl)
                            pts = []
                            for i in range(2):
                                sc = psc.tile([128, 512], F32, tag="sc", name="sc")
                                nc.tensor.matmul(sc[:, lo:hi], ktb[:, 128 * kb:128 * (kb + 1)],
                                                 qtb[h0 + i][:, SC * c + lo:SC * c + hi],
                                                 start=True, stop=True)
                                pt = ppt.tile([128, 512], BF16, tag="pt", name="pt")
                                nc.scalar.activation(pt[:, lo:hi], sc[:, lo:hi], EXP, scale=SCALE)
                                if sl <= 3:
                                    mofs = 128 * sl
                                    nc.vector.tensor_tensor(pt[:, mofs:mofs + 128],
                                                            pt[:, mofs:mofs + 128], mw_t[:], MULT)
                                elif sl >= 8:
                                    mofs = 128 * (sl - 8)
                                    nc.vector.tensor_tensor(pt[:, mofs:mofs + 128],
                                                            pt[:, mofs:mofs + 128], mc_t[:], MULT)
                                pts.append(pt)
                            if prev is not None:
                                emit_pv(*prev)
                            if wo_pending:
                                emit_wo_group(pop, *wo_pending.pop(0))
                            prev = (kb, pts)
                        emit_pv(*prev)
                        if wo_pending:
                            emit_wo_group(pop, *wo_pending.pop(0))
                        for i in range(2):
                            dre = pmisc.tile([1, 512], F32, tag="denr", name="denr")
                            nc.vector.reciprocal(dre[:], dens[i][:])
                            dbc = pmisc.tile([128, 512], F32, tag="denb", name="denb")
                            nc.gpsimd.partition_broadcast(dbc[:], dre[:])
                            at = pat.tile([128, 512], BF16, tag="atb", name="atb")
                            nc.vector.tensor_tensor(at[:], pvs[i][:], dbc[:], MULT)
                            atbs.append(at)
                    while wo_pending:
                        emit_wo_group(pop, *wo_pending.pop(0))
                return atbs

            prev = None
            if fused:
                for c in range(NCH):
                    proj_stage(c)
                    prev = (c, attn_stage(c, prev))
            else:
                for c in range(NCH):
                    proj_stage(c)
                for c in range(NCH):
                    prev = (c, attn_stage(c, prev))
            # final chunk's Wo tail
            with tc.tile_pool(name="outps", bufs=2, space="PSUM") as pop:
                pc, patbs = prev
                for j in range(4):
                    for n in range(8):
                        emit_wo_group(pop, pc, j, n, patbs)


_NC_CACHE = {}


def _build(niter=1, fused=True):
    import os
    fused = os.environ.get("KERNEL_FUSED", "1" if fused else "0") == "1"
    key = (niter, fused)
    if key in _NC_CACHE:
        return _NC_CACHE[key]
    nc = bacc.Bacc("TRN2", target_bir_lowering=False, debug=False,
                   enable_asserts=True, num_devices=NCORES)
    dr = {}

    def din(name, shape, dt=F32):
        dr[name] = nc.dram_tensor(name, shape, dt, kind="ExternalInput").ap()

    din("ht", [HID, S])
    din("wqt", [HID, NQH * D])
    din("wkt", [HID, D])
    din("wvt", [HID, D])
    din("wot", [NQH * D, HID])
    din("ctab", [128, S])
    din("stab", [128, S])
    din("mcaus", [128, 128], BF16)
    din("mwin", [128, 128], BF16)
    out = nc.dram_tensor("out", [S, HID], F32, kind="ExternalOutput").ap()

    with tile.TileContext(nc) as tc:
        _program(tc, dr, out, niter, fused)
    nc.compile()
    _NC_CACHE[key] = nc
    return nc


def make_in_maps(inputs):
    hs = np.asarray(inputs["hidden_states"], dtype=np.float32)
    Wq = np.asarray(inputs["Wq"], dtype=np.float32)
    Wk = np.asarray(inputs["Wk"], dtype=np.float32)
    Wv = np.asarray(inputs["Wv"], dtype=np.float32)
    Wo = np.asarray(inputs["Wo"], dtype=np.float32)
    pos = np.asarray(inputs["position_ids"]).reshape(-1)

    assert hs.shape == (1, S, HID), hs.shape
    H = hs[0]
    HT = np.ascontiguousarray(H.T)

    # RoPE tables in [d%64, s] layout (fp32, mirroring the reference math)
    inv = (1.0 / (ROPE_BASE ** (np.arange(0, D, 2, dtype=np.float32) / D))).astype(np.float32)
    ang = pos.astype(np.float32)[None, :] * inv[:, None]          # [64, S]
    cos64 = np.cos(ang).astype(np.float32)
    sin64 = np.sin(ang).astype(np.float32)
    ctab = np.concatenate([cos64, cos64], axis=0)                 # [128, S]
    stab = np.concatenate([-sin64, sin64], axis=0)                # sign-baked

    kk = np.arange(128)[:, None]
    qq = np.arange(128)[None, :]
    mcaus = (qq >= kk).astype(ml_dtypes.bfloat16)   # causal diag block, [k,q]
    mwin = (qq < kk).astype(ml_dtypes.bfloat16)     # window-edge block, [k,q]

    in_maps = []
    for i in range(NCORES):
        in_maps.append({
            "ht": HT,
            "wqt": np.ascontiguousarray(Wq[512 * i:512 * (i + 1), :].T),
            "wkt": np.ascontiguousarray(Wk[128 * i:128 * (i + 1), :].T),
            "wvt": np.ascontiguousarray(Wv[128 * i:128 * (i + 1), :].T),
            "wot": np.ascontiguousarray(Wo[:, 512 * i:512 * (i + 1)].T),
            "ctab": ctab,
            "stab": stab,
            "mcaus": mcaus,
            "mwin": mwin,
        })

    return in_maps


def kernel(**inputs):
    in_maps = make_in_maps(inputs)
    nc = _build()
    res = run_bass_kernel_spmd(nc, in_maps, core_ids=list(range(NCORES)))

    acc = np.zeros((S, HID), dtype=np.float64)
    for r in res.results:
        acc += r["out"].astype(np.float64)
    return acc.astype(np.float32).reshape(1, S, HID)



# revision 24
# speedup vs baseline: 1.0764x; 1.0764x over previous
"""Trainium2 Bass kernel: Mistral-style GQA attention with sliding-window mask.

Problem: hidden [1,2048,4096] -> Wq/Wk/Wv projections (32 q heads, 8 kv heads,
head_dim 128) -> RoPE -> sliding-window (1024) causal attention -> Wo.

Sharding: tensor-parallel over heads on 8 NeuronCores. Core i owns KV head i
and query heads 4i..4i+3 (Wq/Wk/Wv row-sharded, Wo column-sharded). Each core
computes partial_i = attn_heads_i @ Wo_i^T in HBM; host sums the 8 partials
(the TP all-reduce) to produce the full output.

On-device per core:
  phase A: stream H^T tiles from HBM, cast bf16, matmul into Q^T/K^T/V^T
           (layout [head_dim, seq]), fused RoPE on Q/K out of PSUM,
           V^T transposed to natural [seq, head_dim] via DMA-transpose.
  phase B: block-sparse attention per 512-query chunk: scores^T = K^T-block
           x Q^T-chunk on PE, exp on ACT (PSUM->SBUF bf16), static triangular
           masks on DVE, P@V and row-sum denominators on PE (ones-vector
           matmul), reciprocal+partition_broadcast for normalization, then
           Wo matmuls and fp32 output copy.
"""

import sys

for _p in ("/opt/trn_rl_repo", "/root/.axon_site/_ro/trn_rl_repo"):
    if _p not in sys.path:
        sys.path.insert(0, _p)

import numpy as np
import ml_dtypes

import concourse.bass as bass  # noqa: F401  (registers engine classes)
import concourse.mybir as mybir
import concourse.tile as tile
from concourse import bacc
from concourse.bass_utils import run_bass_kernel_spmd

S = 2048
HID = 4096
D = 128
NQH = 4          # query heads per core
NCORES = 8
SC = 512         # seq chunk
NCH = S // SC
KT = HID // 128  # contraction tiles
WINDOW = 1024
ROPE_BASE = 10000.0
SCALE = 1.0 / float(np.sqrt(D))

F32 = mybir.dt.float32
BF16 = mybir.dt.bfloat16
MULT = mybir.AluOpType.mult
ADD = mybir.AluOpType.add
SUB = mybir.AluOpType.subtract
EXP = mybir.ActivationFunctionType.Exp

# ptb slot layout: slot sl = kb - 4c + 8 for key-block kb in chunk c.
# exp-written region per slot, and statically-zero (memset once) regions.
def _slot_region(sl):
    lo = 128 * (sl - 8) if sl >= 8 else 0
    hi = 512 if sl >= 3 else 128 * (sl + 1)
    return lo, hi

_INVIS = []
for _sl in range(12):
    _lo, _hi = _slot_region(_sl)
    if _lo > 0:
        _INVIS.append((_sl, 0, _lo))
    if _hi < 512:
        _INVIS.append((_sl, _hi, 512))


def _program(tc, dr, out, niter=1, fused=True):
    nc = tc.nc
    ht, wqt, wkt, wvt, wot = dr["ht"], dr["wqt"], dr["wkt"], dr["wvt"], dr["wot"]
    ctab, stab, mcaus, mwin = dr["ctab"], dr["stab"], dr["mcaus"], dr["mwin"]

    def _copy(eng, out_ap, in_ap):
        if eng is nc.scalar:
            eng.copy(out_ap, in_ap)
        else:
            eng.tensor_copy(out_ap, in_ap)

    # Wo PSUM->SBUF copies all ride ACT: DVE must stay clear for the rope
    # chain (a Wo copy queued on DVE behind rope ops holds its PSUM bank and
    # stalls PE's next Wo group)
    def pcopy_rr(out_ap, in_ap):
        _copy(nc.scalar, out_ap, in_ap)

    from contextlib import ExitStack
    if True:
        with ExitStack() as ctx:
            pw = ctx.enter_context(tc.tile_pool(name="persist", bufs=1))
            prt = ctx.enter_context(tc.tile_pool(name="ropet", bufs=2))

            wqb = pw.tile([128, KT * 512], BF16, name="wqb")
            wkb = pw.tile([128, KT * 128], BF16, name="wkb")
            wvb = pw.tile([128, KT * 128], BF16, name="wvb")
            wob = pw.tile([128, NQH * HID], BF16, name="wob")
            qtb = [pw.tile([128, S], BF16, name=f"qtb{h}") for h in range(NQH)]
            ktb = pw.tile([128, S], BF16, name="ktb")
            vtb = pw.tile([128, S], BF16, name="vtb")
            vnat = pw.tile([128, S], BF16, name="vnat")
            cs_t = pw.tile([128, S], F32, name="cs_t")
            sn_t = pw.tile([128, S], F32, name="sn_t")
            mc_t = pw.tile([128, 128], BF16, name="mc_t")
            mw_t = pw.tile([128, 128], BF16, name="mw_t")
            ones_t = pw.tile([128, 1], BF16, name="ones_t")
            # q2/q3 staging, pre-rotated: [pre(512), rot(512)] per head
            psq = pw.tile([128, 2048], BF16, name="psq")

            nc.gpsimd.memset(ones_t[:], 1.0)

            def _rope_staged(dst, pre, rot, c, co, w):
                """staged rope: pre/rot already in SBUF (rot = half-swapped),
                full-width same-base-partition ops (SBUF+SBUF DVE operands
                must share a base partition)."""
                lo, hi = SC * c + co, SC * c + co + w
                t1 = prt.tile([128, w], F32, tag="rt1", name="st1")
                t2 = prt.tile([128, w], F32, tag="rt2", name="st2")
                nc.vector.tensor_tensor(t1[:], pre, cs_t[:, lo:hi], MULT)
                nc.vector.tensor_tensor(t2[:], rot, sn_t[:, lo:hi], MULT)
                nc.vector.tensor_tensor(dst, t1[:], t2[:], ADD)

            def _rope(dst, p, c, co, w, eng="dve"):
                """dst[bf16 [128,w] slice] = rope(p [[128,w] slice, PSUM or SBUF]),
                chunk c col-offset co.

                cs_t is cos duplicated across both halves; sn_t is sign-baked
                sin: rows 0:64 = -sin, rows 64:128 = +sin, so
                out = q*cos + rot(q)*sn with rot a plain half-swap.
                """
                use_gp = eng == "gp"
                lo, hi = SC * c + co, SC * c + co + w
                csl = cs_t[:, lo:hi]
                snl = sn_t[:, lo:hi]
                if use_gp:
                    pre = prt.tile([128, w], F32, tag="rpre", name="rpre", bufs=1)
                    rot = prt.tile([128, w], F32, tag="rrot", name="rrot", bufs=1)
                    nc.scalar.copy(pre[:], p[:])
                    nc.scalar.copy(rot[0:64, :], p[64:128, :])
                    nc.scalar.copy(rot[64:128, :], p[0:64, :])
                    g1 = prt.tile([128, w], F32, tag="rt1", name="g1")
                    g2 = prt.tile([128, w], F32, tag="rt2", name="g2")
                    nc.gpsimd.tensor_tensor(g1[:], pre[:], csl, MULT)
                    nc.gpsimd.tensor_tensor(g2[:], rot[:], snl, MULT)
                    nc.gpsimd.tensor_tensor(dst[:, :], g1[:], g2[:], ADD)
                    return
                e = nc.vector
                t1 = prt.tile([64, w], F32, tag="rt1", name="rt1")
                t2 = prt.tile([64, w], F32, tag="rt2", name="rt2")
                e.tensor_tensor(t1[:], p[0:64, :], csl[0:64, :], MULT)
                e.tensor_tensor(t2[:], p[64:128, :], snl[0:64, :], MULT)
                e.tensor_tensor(dst[0:64, :], t1[:], t2[:], ADD)
                t3 = prt.tile([64, w], F32, tag="rt1", name="rt3")
                t4 = prt.tile([64, w], F32, tag="rt2", name="rt4")
                e.tensor_tensor(t3[:], p[64:128, :], csl[64:128, :], MULT)
                e.tensor_tensor(t4[:], p[0:64, :], snl[64:128, :], MULT)
                e.tensor_tensor(dst[64:128, :], t3[:], t4[:], ADD)

            phb = ctx.enter_context(tc.tile_pool(name="htbp", bufs=12))
            ppt = ctx.enter_context(tc.tile_pool(name="ptp", bufs=7))
            pmisc = ctx.enter_context(tc.tile_pool(name="miscb", bufs=2))
            pat = ctx.enter_context(tc.tile_pool(name="atbp", bufs=8))
            posb = ctx.enter_context(tc.tile_pool(name="osbp", bufs=2))
            # Wo-output PSUM pool stays open across proj+attn of every chunk:
            # 2 banks here + 6 proj banks = 8; 2 + 6 attn banks = 8.
            pop = ctx.enter_context(tc.tile_pool(name="outps", bufs=2, space="PSUM"))

            # Wo emission: one group = one [128q, 512hid] output tile of the
            # PREVIOUS chunk; groups interleave into the proj k-loop (PSUM
            # budget) and the attn pre-loop (covers the rope-tail window).
            wo_state = {"c": None, "atbs": None, "m": 0, "obig": None}

            def emit_wo_group():
                st = wo_state
                if st["c"] is None or st["m"] >= 32:
                    return False
                wj, wn = st["m"] // 8, st["m"] % 8
                wc, watbs = st["c"], st["atbs"]
                if wn % 4 == 0:
                    st["obig"] = posb.tile([128, HID // 2], BF16, tag="osb", name="osb")
                po = pop.tile([128, 512], F32, tag="po", name="po")
                for h in range(NQH):
                    nc.tensor.matmul(po[:], watbs[h][:, 128 * wj:128 * (wj + 1)],
                                     wob[:, HID * h + 512 * wn:HID * h + 512 * (wn + 1)],
                                     start=(h == 0), stop=(h == NQH - 1))
                pcopy_rr(st["obig"][:, 512 * (wn % 4):512 * (wn % 4 + 1)], po[:])
                if wn % 4 == 3:
                    eng = nc.scalar if wn < 4 else nc.gpsimd
                    eng.dma_start(out[SC * wc + 128 * wj:SC * wc + 128 * (wj + 1),
                                      2048 * (wn // 4):2048 * (wn // 4 + 1)],
                                  st["obig"][:])
                st["m"] += 1
                return True

            def proj_stage(c):
                # projections for chunk c, in two 256-col seq halves: rope of
                # half 0 runs on DVE/GP/ACT underneath PE's half-1 k-loop, so
                # only the half-1 rope tail is exposed at the chunk boundary
                # (the attn PSUM pools can only open once the proj pool closes,
                # i.e. after the last rope drains its bank). Chunk 0 stays
                # full-width — its k-loop must cover the serial weight-DMA
                # stream — and stages q2/q3 through SBUF so the pool close
                # only waits on the K/q0/q1 ropes.
                halves = 1 if c == 0 else 2
                w = SC // halves
                with tc.tile_pool(name="projps", bufs=6, space="PSUM") as ppp:
                    ps6 = [ppp.tile([128, 512], F32, tag="proj", name=f"proj{c}_{i}")
                           for i in range(6)]
                    it = 0
                    for half in range(halves):
                        co = w * half
                        hb = None
                        for k in range(KT):
                            g = k // 4
                            if k % 4 == 0:
                                hb = phb.tile([128, 4 * w], BF16,
                                              tag=f"htb{halves}", name="hb",
                                              bufs=(5 if halves == 1 else 12))
                                nc.sync.dma_start(
                                    hb[:].rearrange("p (k j) -> p k j", j=w),
                                    ht[512 * g:512 * (g + 1),
                                       SC * c + co:SC * c + co + w].rearrange(
                                        "(k p) j -> p k j", p=128))
                                if c == 0 and g < 4:
                                    nc.sync.dma_start(
                                        wqb[:, 4096 * g:4096 * (g + 1)].rearrange(
                                            "p (k j) -> p k j", j=512),
                                        wqt[1024 * g:1024 * (g + 1), :].rearrange(
                                            "(k p) j -> p k j", p=128))
                                    nc.sync.dma_start(
                                        wkb[:, 1024 * g:1024 * (g + 1)].rearrange(
                                            "p (k j) -> p k j", j=128),
                                        wkt[1024 * g:1024 * (g + 1), :].rearrange(
                                            "(k p) j -> p k j", p=128))
                                    nc.sync.dma_start(
                                        wvb[:, 1024 * g:1024 * (g + 1)].rearrange(
                                            "p (k j) -> p k j", j=128),
                                        wvt[1024 * g:1024 * (g + 1), :].rearrange(
                                            "(k p) j -> p k j", p=128))
                                if c == 0 and g == 2:
                                    nc.sync.dma_start(cs_t[:, 0:SC], ctab[:, 0:SC])
                                    nc.sync.dma_start(sn_t[:, 0:SC], stab[:, 0:SC])
                                    nc.sync.dma_start(mc_t[:], mcaus[:])
                                    nc.sync.dma_start(mw_t[:], mwin[:])
                                if c == 0 and g >= 4:
                                    # wob halves ride the c0 tail (the weight
                                    # stream has drained by k=16)
                                    wh = 2 * (g - 4)
                                    for j in (wh, wh + 1):
                                        nc.sync.dma_start(
                                            wob[:, 2048 * j:2048 * (j + 1)],
                                            wot[128 * (j // 2):128 * (j // 2 + 1),
                                                4096 * (j % 2) // 2:
                                                4096 * (j % 2) // 2 + 2048])
                                if c == 1 and half == 0 and g == 0:
                                    nc.sync.dma_start(cs_t[:, SC:], ctab[:, SC:])
                                    nc.sync.dma_start(sn_t[:, SC:], stab[:, SC:])
                            hsl = hb[:, w * (k % 4):w * (k % 4 + 1)]
                            first, last = k == 0, k == KT - 1
                            for h in range(NQH):
                                nc.tensor.matmul(
                                    ps6[h][:, co:co + w],
                                    wqb[:, 512 * k + 128 * h:512 * k + 128 * (h + 1)],
                                    hsl, start=first, stop=last, skip_group_check=True)
                            nc.tensor.matmul(ps6[4][:, co:co + w],
                                             wkb[:, 128 * k:128 * (k + 1)], hsl,
                                             start=first, stop=last, skip_group_check=True)
                            nc.tensor.matmul(ps6[5][:, co:co + w],
                                             wvb[:, 128 * k:128 * (k + 1)], hsl,
                                             start=first, stop=last, skip_group_check=True)
                            if it >= 8 and wo_state["m"] < 26:
                                emit_wo_group()
                            it += 1
                        for r0 in range(0, w, 256):
                            _rope(ktb[:, SC * c + co + r0:SC * c + co + r0 + 256],
                                  ps6[4][:, co + r0:co + r0 + 256], c, co + r0, 256,
                                  eng="dve")
                            _rope(qtb[0][:, SC * c + co + r0:SC * c + co + r0 + 256],
                                  ps6[0][:, co + r0:co + r0 + 256], c, co + r0, 256,
                                  eng="gp")
                            _rope(qtb[1][:, SC * c + co + r0:SC * c + co + r0 + 256],
                                  ps6[1][:, co + r0:co + r0 + 256], c, co + r0, 256,
                                  eng="dve")
                        if halves == 1:
                            # free the q2/q3 banks early: stage to SBUF via
                            # ACT (rot half pre-swapped), rope later (pair 1
                            # needs them ~10us on)
                            for qi in (2, 3):
                                b = 1024 * (qi - 2)
                                nc.scalar.copy(psq[:, b:b + 512], ps6[qi][:])
                                nc.scalar.copy(psq[0:64, b + 512:b + 1024],
                                               ps6[qi][64:128, :])
                                nc.scalar.copy(psq[64:128, b + 512:b + 1024],
                                               ps6[qi][0:64, :])
                        else:
                            _rope(qtb[2][:, SC * c + co:SC * c + co + w],
                                  ps6[2][:, co:co + w], c, co, w, eng="dve")
                            _rope(qtb[3][:, SC * c + co:SC * c + co + w],
                                  ps6[3][:, co:co + w], c, co, w, eng="dve")
                        nc.scalar.copy(vtb[:, SC * c + co:SC * c + co + w],
                                       ps6[5][:, co:co + w])
                        if halves == 1:
                            for r0 in range(0, SC, 256):
                                _rope_staged(
                                    qtb[2][:, SC * c + r0:SC * c + r0 + 256],
                                    psq[:, r0:r0 + 256],
                                    psq[:, 512 + r0:512 + r0 + 256], c, r0, 256)
                                _rope_staged(
                                    qtb[3][:, SC * c + r0:SC * c + r0 + 256],
                                    psq[:, 1024 + r0:1024 + r0 + 256],
                                    psq[:, 1536 + r0:1536 + r0 + 256], c, r0, 256)
                        for b2 in range(w // 128):
                            bo = 128 * ((SC // 128) * c + (w // 128) * half + b2)
                            nc.scalar.dma_start_transpose(
                                vnat[:, bo:bo + 128],
                                vtb[:, SC * c + co + 128 * b2:SC * c + co + 128 * (b2 + 1)])

            def attn_stage(c):
                # block-sparse attention for chunk c (past K/V only: sliding
                # window); leftover Wo groups of chunk c-1 fill the rope tail.
                with tc.tile_pool(name="scps", bufs=2, space="PSUM") as psc, \
                     tc.tile_pool(name="pvps", bufs=3, space="PSUM") as ppv, \
                     tc.tile_pool(name="denps", bufs=1, space="PSUM") as pdn:
                    kbs = list(range(max(0, 4 * c - 8), 4 * c + 4))
                    first_kb, last_kb = kbs[0], kbs[-1]
                    while emit_wo_group():
                        pass
                    atbs = []
                    for h0 in range(0, NQH, 2):
                        # process a PAIR of heads per key-block sweep: two
                        # independent score/exp chains per step keep ACT fed.
                        pvs = [ppv.tile([128, 512], F32, tag="pv", name="pv")
                               for _ in range(2)]
                        # one PSUM bank holds both heads' denominator rows
                        # (matmul out base partition must be 0/32/64)
                        pdnt = pdn.tile([33, 512], F32, tag="den", name="den")
                        dens = [pdnt[0:1, :], pdnt[32:33, :]]

                        def emit_pv(kb, pts):
                            # accumulate P@V and row-sums over exact visible slices.
                            sl = kb - 4 * c + 8
                            lo, hi = _slot_region(sl)
                            vsl = vnat[:, 128 * kb:128 * (kb + 1)]
                            for i in range(2):
                                nc.tensor.matmul(pvs[i][:, lo:hi], vsl, pts[i][:, lo:hi],
                                                 start=(kb == first_kb), stop=(kb == last_kb),
                                                 skip_group_check=True)
                                nc.tensor.matmul(dens[i][:, lo:hi], ones_t[:], pts[i][:, lo:hi],
                                                 start=(kb == first_kb), stop=(kb == last_kb),
                                                 skip_group_check=True)

                        pending = []
                        for kb in kbs:
                            sl = kb - 4 * c + 8
                            lo, hi = _slot_region(sl)
                            pts = []
                            for i in range(2):
                                sc = psc.tile([128, 512], F32, tag="sc", name="sc")
                                nc.tensor.matmul(sc[:, lo:hi], ktb[:, 128 * kb:128 * (kb + 1)],
                                                 qtb[h0 + i][:, SC * c + lo:SC * c + hi],
                                                 start=True, stop=True)
                                pt = ppt.tile([128, 512], BF16, tag="pt", name="pt")
                                nc.scalar.activation(pt[:, lo:hi], sc[:, lo:hi], EXP, scale=SCALE)
                                if sl <= 3:
                                    mofs = 128 * sl
                                    nc.vector.tensor_tensor(pt[:, mofs:mofs + 128],
                                                            pt[:, mofs:mofs + 128], mw_t[:], MULT)
                                elif sl >= 8:
                                    mofs = 128 * (sl - 8)
                                    nc.vector.tensor_tensor(pt[:, mofs:mofs + 128],
                                                            pt[:, mofs:mofs + 128], mc_t[:], MULT)
                                pts.append(pt)
                            pending.append((kb, pts))
                            if len(pending) > 1:
                                emit_pv(*pending.pop(0))
                        for pv_item in pending:
                            emit_pv(*pv_item)
                        for i in range(2):
                            # stage P@V out of PSUM via ACT so the bank frees
                            # immediately; normalize from SBUF off the
                            # critical path
                            pvu = pat.tile([128, 512], BF16, tag="pvu",
                                           name="pvu", bufs=2)
                            nc.scalar.copy(pvu[:], pvs[i][:])
                            dre = pmisc.tile([1, 512], BF16, tag="denr", name="denr")
                            with nc.allow_low_precision(reason="softmax denom to bf16"):
                                nc.vector.reciprocal(dre[:], dens[i])
                            dbc = pmisc.tile([128, 512], BF16, tag="denb", name="denb")
                            nc.gpsimd.partition_broadcast(dbc[:], dre[:])
                            at = pat.tile([128, 512], BF16, tag="atb", name="atb")
                            nc.vector.tensor_tensor(at[:], pvu[:], dbc[:], MULT)
                            atbs.append(at)
                return atbs

            for _it in range(niter):
                for c in range(NCH):
                    proj_stage(c)
                    atbs = attn_stage(c)
                    wo_state.update(c=c, atbs=atbs, m=0, obig=None)
            # the last chunk's Wo groups of each iteration ride the next
            # iteration's proj(0)/attn(0); only the final one drains here
            while emit_wo_group():
                pass


_NC_CACHE = {}


def _build(niter=1, fused=True):
    import os
    fused = os.environ.get("KERNEL_FUSED", "1" if fused else "0") == "1"
    key = (niter, fused)
    if key in _NC_CACHE:
        return _NC_CACHE[key]
    nc = bacc.Bacc("TRN2", target_bir_lowering=False, debug=False,
                   enable_asserts=True, num_devices=NCORES)
    dr = {}

    def din(name, shape, dt=F32):
        dr[name] = nc.dram_tensor(name, shape, dt, kind="ExternalInput").ap()

    din("ht", [HID, S], BF16)
    din("wqt", [HID, NQH * D], BF16)
    din("wkt", [HID, D], BF16)
    din("wvt", [HID, D], BF16)
    din("wot", [NQH * D, HID], BF16)
    din("ctab", [128, S])
    din("stab", [128, S])
    din("mcaus", [128, 128], BF16)
    din("mwin", [128, 128], BF16)
    out = nc.dram_tensor("out", [S, HID], BF16, kind="ExternalOutput").ap()

    with tile.TileContext(nc) as tc:
        _program(tc, dr, out, niter, fused)
    nc.compile()
    _NC_CACHE[key] = nc
    return nc


def make_in_maps(inputs):
    hs = np.asarray(inputs["hidden_states"], dtype=np.float32)
    Wq = np.asarray(inputs["Wq"], dtype=np.float32)
    Wk = np.asarray(inputs["Wk"], dtype=np.float32)
    Wv = np.asarray(inputs["Wv"], dtype=np.float32)
    Wo = np.asarray(inputs["Wo"], dtype=np.float32)
    pos = np.asarray(inputs["position_ids"]).reshape(-1)

    assert hs.shape == (1, S, HID), hs.shape
    H = hs[0]
    HT = np.ascontiguousarray(H.T)

    # RoPE tables in [d%64, s] layout (fp32, mirroring the reference math)
    inv = (1.0 / (ROPE_BASE ** (np.arange(0, D, 2, dtype=np.float32) / D))).astype(np.float32)
    ang = pos.astype(np.float32)[None, :] * inv[:, None]          # [64, S]
    cos64 = np.cos(ang).astype(np.float32)
    sin64 = np.sin(ang).astype(np.float32)
    ctab = np.concatenate([cos64, cos64], axis=0)                 # [128, S]
    stab = np.concatenate([-sin64, sin64], axis=0)                # sign-baked

    kk = np.arange(128)[:, None]
    qq = np.arange(128)[None, :]
    mcaus = (qq >= kk).astype(ml_dtypes.bfloat16)   # causal diag block, [k,q]
    mwin = (qq < kk).astype(ml_dtypes.bfloat16)     # window-edge block, [k,q]

    BF = ml_dtypes.bfloat16
    HTB = HT.astype(BF)
    in_maps = []
    for i in range(NCORES):
        in_maps.append({
            "ht": HTB,
            "wqt": np.ascontiguousarray(Wq[512 * i:512 * (i + 1), :].T).astype(BF),
            "wkt": np.ascontiguousarray(Wk[128 * i:128 * (i + 1), :].T).astype(BF),
            "wvt": np.ascontiguousarray(Wv[128 * i:128 * (i + 1), :].T).astype(BF),
            "wot": np.ascontiguousarray(Wo[:, 512 * i:512 * (i + 1)].T).astype(BF),
            "ctab": ctab,
            "stab": stab,
            "mcaus": mcaus,
            "mwin": mwin,
        })

    return in_maps


def kernel(**inputs):
    in_maps = make_in_maps(inputs)
    nc = _build()
    res = run_bass_kernel_spmd(nc, in_maps, core_ids=list(range(NCORES)))

    acc = np.zeros((S, HID), dtype=np.float32)
    for r in res.results:
        acc += r["out"].astype(np.float32)
    return acc.reshape(1, S, HID)



# revision 34
# speedup vs baseline: 1.2549x; 1.1659x over previous
"""Trainium2 Bass kernel: Mistral-style GQA attention with sliding-window mask.

Problem: hidden [1,2048,4096] -> Wq/Wk/Wv projections (32 q heads, 8 kv heads,
head_dim 128) -> RoPE -> sliding-window (1024) causal attention -> Wo.

Sharding: tensor-parallel over heads on 8 NeuronCores. Core i owns KV head i
and query heads 4i..4i+3 (Wq/Wk/Wv row-sharded, Wo column-sharded). Each core
computes partial_i = attn_heads_i @ Wo_i^T in HBM; host sums the 8 partials
(the TP all-reduce) to produce the full output.

On-device per core:
  phase A: stream H^T tiles from HBM, cast bf16, matmul into Q^T/K^T/V^T
           (layout [head_dim, seq]), fused RoPE on Q/K out of PSUM,
           V^T transposed to natural [seq, head_dim] via DMA-transpose.
  phase B: block-sparse attention per 512-query chunk: scores^T = K^T-block
           x Q^T-chunk on PE, exp on ACT (PSUM->SBUF bf16), static triangular
           masks on DVE, P@V and row-sum denominators on PE (ones-vector
           matmul), reciprocal+partition_broadcast for normalization, then
           Wo matmuls and fp32 output copy.
"""

import sys

for _p in ("/opt/trn_rl_repo", "/root/.axon_site/_ro/trn_rl_repo"):
    if _p not in sys.path:
        sys.path.insert(0, _p)

import numpy as np
import ml_dtypes

import concourse.bass as bass  # noqa: F401  (registers engine classes)
import concourse.mybir as mybir
import concourse.tile as tile
from concourse import bacc
from concourse.bass_utils import run_bass_kernel_spmd

S = 2048
HID = 4096
D = 128
NQH = 4          # query heads per core
NCORES = 8
SC = 512         # seq chunk
NCH = S // SC
KT = HID // 128  # contraction tiles
WINDOW = 1024
ROPE_BASE = 10000.0
SCALE = 1.0 / float(np.sqrt(D))

F32 = mybir.dt.float32
BF16 = mybir.dt.bfloat16
MULT = mybir.AluOpType.mult
ADD = mybir.AluOpType.add
SUB = mybir.AluOpType.subtract
EXP = mybir.ActivationFunctionType.Exp

# ptb slot layout: slot sl = kb - 4c + 8 for key-block kb in chunk c.
# exp-written region per slot, and statically-zero (memset once) regions.
def _slot_region(sl):
    lo = 128 * (sl - 8) if sl >= 8 else 0
    hi = 512 if sl >= 3 else 128 * (sl + 1)
    return lo, hi

_INVIS = []
for _sl in range(12):
    _lo, _hi = _slot_region(_sl)
    if _lo > 0:
        _INVIS.append((_sl, 0, _lo))
    if _hi < 512:
        _INVIS.append((_sl, _hi, 512))


def _program(tc, dr, out, niter=1, fused=True):
    nc = tc.nc
    ht, wqt, wkt, wvt, wot = dr["ht"], dr["wqt"], dr["wkt"], dr["wvt"], dr["wot"]
    ctab, stab, mcaus, mwin = dr["ctab"], dr["stab"], dr["mcaus"], dr["mwin"]

    def _copy(eng, out_ap, in_ap):
        if eng is nc.scalar:
            eng.copy(out_ap, in_ap)
        else:
            eng.tensor_copy(out_ap, in_ap)

    # Wo PSUM->SBUF copies: DVE mid-chunk (it is idle there), ACT for the
    # boundary groups (DVE ropes then; a Wo copy queued behind rope ops
    # holds its PSUM bank and stalls PE's next Wo group)
    def pcopy_rr(out_ap, in_ap, eng=None):
        _copy(eng or nc.scalar, out_ap, in_ap)

    from contextlib import ExitStack
    if True:
        with ExitStack() as ctx:
            pw = ctx.enter_context(tc.tile_pool(name="persist", bufs=1))
            prt = ctx.enter_context(tc.tile_pool(name="ropet", bufs=2))

            wqb = pw.tile([128, KT * 512], BF16, name="wqb")
            wkb = pw.tile([128, KT * 128], BF16, name="wkb")
            wvb = pw.tile([128, KT * 128], BF16, name="wvb")
            wob = pw.tile([128, NQH * HID], BF16, name="wob")
            qtb = [pw.tile([128, S], BF16, name=f"qtb{h}") for h in range(NQH)]
            ktb = pw.tile([128, S], BF16, name="ktb")
            vtb = pw.tile([128, S], BF16, name="vtb")
            vnat = pw.tile([128, S], BF16, name="vnat")
            cs_t = pw.tile([128, S], F32, name="cs_t")
            sn_t = pw.tile([128, S], F32, name="sn_t")
            mc_t = pw.tile([128, 128], BF16, name="mc_t")
            mw_t = pw.tile([128, 128], BF16, name="mw_t")
            ones_t = pw.tile([128, 1], BF16, name="ones_t")
            # q2/q3 staging, pre-rotated: [pre(512), rot(512)] per head
            psq = pw.tile([128, 2048], BF16, name="psq")

            nc.gpsimd.memset(ones_t[:], 1.0)

            def _rope_staged(dst, pre, rot, c, co, w):
                """staged rope: pre/rot already in SBUF (rot = half-swapped),
                full-width same-base-partition ops (SBUF+SBUF DVE operands
                must share a base partition)."""
                lo, hi = SC * c + co, SC * c + co + w
                t1 = prt.tile([128, w], F32, tag="rt1", name="st1")
                t2 = prt.tile([128, w], F32, tag="rt2", name="st2")
                nc.vector.tensor_tensor(t1[:], pre, cs_t[:, lo:hi], MULT)
                nc.vector.tensor_tensor(t2[:], rot, sn_t[:, lo:hi], MULT)
                nc.vector.tensor_tensor(dst, t1[:], t2[:], ADD)

            def _rope(dst, p, c, co, w, eng="dve"):
                """dst[bf16 [128,w] slice] = rope(p [[128,w] slice, PSUM or SBUF]),
                chunk c col-offset co.

                cs_t is cos duplicated across both halves; sn_t is sign-baked
                sin: rows 0:64 = -sin, rows 64:128 = +sin, so
                out = q*cos + rot(q)*sn with rot a plain half-swap.
                """
                use_gp = eng == "gp"
                lo, hi = SC * c + co, SC * c + co + w
                csl = cs_t[:, lo:hi]
                snl = sn_t[:, lo:hi]
                if use_gp:
                    pre = prt.tile([128, w], BF16, tag="rpre", name="rpre", bufs=2)
                    rot = prt.tile([128, w], BF16, tag="rrot", name="rrot", bufs=2)
                    nc.scalar.copy(pre[:], p[:])
                    nc.scalar.copy(rot[0:64, :], p[64:128, :])
                    nc.scalar.copy(rot[64:128, :], p[0:64, :])
                    g1 = prt.tile([128, w], F32, tag="rt1", name="g1")
                    g2 = prt.tile([128, w], F32, tag="rt2", name="g2")
                    nc.gpsimd.tensor_tensor(g1[:], pre[:], csl, MULT)
                    nc.gpsimd.tensor_tensor(g2[:], rot[:], snl, MULT)
                    nc.gpsimd.tensor_tensor(dst[:, :], g1[:], g2[:], ADD)
                    return
                e = nc.vector
                t1 = prt.tile([64, w], F32, tag="rt1", name="rt1")
                t2 = prt.tile([64, w], F32, tag="rt2", name="rt2")
                e.tensor_tensor(t1[:], p[0:64, :], csl[0:64, :], MULT)
                e.tensor_tensor(t2[:], p[64:128, :], snl[0:64, :], MULT)
                e.tensor_tensor(dst[0:64, :], t1[:], t2[:], ADD)
                t3 = prt.tile([64, w], F32, tag="rt1", name="rt3")
                t4 = prt.tile([64, w], F32, tag="rt2", name="rt4")
                e.tensor_tensor(t3[:], p[64:128, :], csl[64:128, :], MULT)
                e.tensor_tensor(t4[:], p[0:64, :], snl[64:128, :], MULT)
                e.tensor_tensor(dst[64:128, :], t3[:], t4[:], ADD)

            phb = ctx.enter_context(tc.tile_pool(name="htbp", bufs=12))
            ppt = ctx.enter_context(tc.tile_pool(name="ptp", bufs=7))
            pmisc = ctx.enter_context(tc.tile_pool(name="miscb", bufs=2))
            pat = ctx.enter_context(tc.tile_pool(name="atbp", bufs=8))
            posb = ctx.enter_context(tc.tile_pool(name="osbp", bufs=2))
            # Wo-output PSUM pool stays open across proj+attn of every chunk:
            # 2 banks here + 6 proj banks = 8; 2 + 6 attn banks = 8.
            pop = ctx.enter_context(tc.tile_pool(name="outps", bufs=2, space="PSUM"))

            # Wo emission: one group = one [128q, 512hid] output tile of the
            # PREVIOUS chunk; groups interleave into the proj k-loop (PSUM
            # budget) and the attn pre-loop (covers the rope-tail window).
            wo_state = {"c": None, "atbs": None, "m": 0, "obig": None}

            def emit_wo_group():
                st = wo_state
                if st["c"] is None or st["m"] >= 32:
                    return False
                wj, wn = st["m"] // 8, st["m"] % 8
                wc, watbs = st["c"], st["atbs"]
                if wn % 4 == 0:
                    st["obig"] = posb.tile([128, HID // 2], BF16, tag="osb", name="osb")
                po = pop.tile([128, 512], F32, tag="po", name="po")
                for h in range(NQH):
                    nc.tensor.matmul(po[:], watbs[h][:, 128 * wj:128 * (wj + 1)],
                                     wob[:, HID * h + 512 * wn:HID * h + 512 * (wn + 1)],
                                     start=(h == 0), stop=(h == NQH - 1))
                pcopy_rr(st["obig"][:, 512 * (wn % 4):512 * (wn % 4 + 1)], po[:],
                         eng=(nc.vector if st["m"] < 24 else nc.scalar))
                if wn % 4 == 3:
                    nc.scalar.dma_start(
                        out[SC * wc + 128 * wj:SC * wc + 128 * (wj + 1),
                            2048 * (wn // 4):2048 * (wn // 4 + 1)],
                        st["obig"][:])
                st["m"] += 1
                return True

            def proj_stage(c):
                # projections for chunk c, in two 256-col seq halves: rope of
                # half 0 runs on DVE/GP/ACT underneath PE's half-1 k-loop, so
                # only the half-1 rope tail is exposed at the chunk boundary
                # (the attn PSUM pools can only open once the proj pool closes,
                # i.e. after the last rope drains its bank). Chunk 0 stays
                # full-width — its k-loop must cover the serial weight-DMA
                # stream — and stages q2/q3 through SBUF so the pool close
                # only waits on the K/q0/q1 ropes.
                halves = 1 if c == 0 else 2
                w = SC // halves
                with tc.tile_pool(name="projps", bufs=6, space="PSUM") as ppp:
                    ps6 = [ppp.tile([128, 512], F32, tag="proj", name=f"proj{c}_{i}")
                           for i in range(6)]
                    it = 0
                    for half in range(halves):
                        co = w * half
                        hb = None
                        for k in range(KT):
                            g = k // 4
                            if k % 4 == 0:
                                hb = phb.tile([128, 4 * w], BF16,
                                              tag=f"htb{halves}", name="hb",
                                              bufs=(5 if halves == 1 else 12))
                                nc.sync.dma_start(
                                    hb[:].rearrange("p (k j) -> p k j", j=w),
                                    ht[512 * g:512 * (g + 1),
                                       SC * c + co:SC * c + co + w].rearrange(
                                        "(k p) j -> p k j", p=128))
                                if c == 0 and g < 4:
                                    nc.sync.dma_start(
                                        wqb[:, 4096 * g:4096 * (g + 1)].rearrange(
                                            "p (k j) -> p k j", j=512),
                                        wqt[1024 * g:1024 * (g + 1), :].rearrange(
                                            "(k p) j -> p k j", p=128))
                                    nc.sync.dma_start(
                                        wkb[:, 1024 * g:1024 * (g + 1)].rearrange(
                                            "p (k j) -> p k j", j=128),
                                        wkt[1024 * g:1024 * (g + 1), :].rearrange(
                                            "(k p) j -> p k j", p=128))
                                    nc.sync.dma_start(
                                        wvb[:, 1024 * g:1024 * (g + 1)].rearrange(
                                            "p (k j) -> p k j", j=128),
                                        wvt[1024 * g:1024 * (g + 1), :].rearrange(
                                            "(k p) j -> p k j", p=128))
                                if c == 0 and g == 2:
                                    nc.sync.dma_start(cs_t[:, 0:SC], ctab[:, 0:SC])
                                    nc.sync.dma_start(sn_t[:, 0:SC], stab[:, 0:SC])
                                    nc.sync.dma_start(mc_t[:], mcaus[:])
                                    nc.sync.dma_start(mw_t[:], mwin[:])
                                if c == 0 and g >= 4:
                                    # wob halves ride the c0 tail (the weight
                                    # stream has drained by k=16)
                                    wh = 2 * (g - 4)
                                    for j in (wh, wh + 1):
                                        nc.sync.dma_start(
                                            wob[:, 2048 * j:2048 * (j + 1)],
                                            wot[128 * (j // 2):128 * (j // 2 + 1),
                                                4096 * (j % 2) // 2:
                                                4096 * (j % 2) // 2 + 2048])
                                if c == 1 and half == 0 and g == 0:
                                    nc.sync.dma_start(cs_t[:, SC:], ctab[:, SC:])
                                    nc.sync.dma_start(sn_t[:, SC:], stab[:, SC:])
                            hsl = hb[:, w * (k % 4):w * (k % 4 + 1)]
                            first, last = k == 0, k == KT - 1
                            for h in range(NQH):
                                nc.tensor.matmul(
                                    ps6[h][:, co:co + w],
                                    wqb[:, 512 * k + 128 * h:512 * k + 128 * (h + 1)],
                                    hsl, start=first, stop=last, skip_group_check=True)
                            nc.tensor.matmul(ps6[4][:, co:co + w],
                                             wkb[:, 128 * k:128 * (k + 1)], hsl,
                                             start=first, stop=last, skip_group_check=True)
                            nc.tensor.matmul(ps6[5][:, co:co + w],
                                             wvb[:, 128 * k:128 * (k + 1)], hsl,
                                             start=first, stop=last, skip_group_check=True)
                            if it >= 8 and wo_state["m"] < 24:
                                emit_wo_group()
                            it += 1
                        if half == halves - 1:
                            # their ACT copies precede the rope pre-copies in
                            # ACT's in-order queue, keeping the po-bank ring
                            # turning while DVE ropes
                            emit_wo_group()
                            emit_wo_group()
                        for r0 in range(0, w, 256):
                            # q1 before K on DVE: the attn score pool reuses
                            # the q0/q1 PSUM banks, so drain those first (q0
                            # frees via its ACT pre-copies)
                            _rope(qtb[0][:, SC * c + co + r0:SC * c + co + r0 + 256],
                                  ps6[0][:, co + r0:co + r0 + 256], c, co + r0, 256,
                                  eng="gp")
                            _rope(qtb[1][:, SC * c + co + r0:SC * c + co + r0 + 256],
                                  ps6[1][:, co + r0:co + r0 + 256], c, co + r0, 256,
                                  eng="dve")
                            _rope(ktb[:, SC * c + co + r0:SC * c + co + r0 + 256],
                                  ps6[4][:, co + r0:co + r0 + 256], c, co + r0, 256,
                                  eng="dve")
                        if halves == 1:
                            # free the q2/q3 banks early: stage to SBUF via
                            # ACT (rot half pre-swapped), rope later (pair 1
                            # needs them ~10us on)
                            for qi in (2, 3):
                                b = 1024 * (qi - 2)
                                nc.scalar.copy(psq[:, b:b + 512], ps6[qi][:])
                                nc.scalar.copy(psq[0:64, b + 512:b + 1024],
                                               ps6[qi][64:128, :])
                                nc.scalar.copy(psq[64:128, b + 512:b + 1024],
                                               ps6[qi][0:64, :])
                        else:
                            _rope(qtb[2][:, SC * c + co:SC * c + co + w],
                                  ps6[2][:, co:co + w], c, co, w, eng="dve")
                            _rope(qtb[3][:, SC * c + co:SC * c + co + w],
                                  ps6[3][:, co:co + w], c, co, w, eng="dve")
                        nc.scalar.copy(vtb[:, SC * c + co:SC * c + co + w],
                                       ps6[5][:, co:co + w])
                        if halves == 1:
                            for r0 in range(0, SC, 256):
                                _rope_staged(
                                    qtb[2][:, SC * c + r0:SC * c + r0 + 256],
                                    psq[:, r0:r0 + 256],
                                    psq[:, 512 + r0:512 + r0 + 256], c, r0, 256)
                                _rope_staged(
                                    qtb[3][:, SC * c + r0:SC * c + r0 + 256],
                                    psq[:, 1024 + r0:1024 + r0 + 256],
                                    psq[:, 1536 + r0:1536 + r0 + 256], c, r0, 256)
                        for b2 in range(w // 128):
                            bo = 128 * ((SC // 128) * c + (w // 128) * half + b2)
                            nc.scalar.dma_start_transpose(
                                vnat[:, bo:bo + 128],
                                vtb[:, SC * c + co + 128 * b2:SC * c + co + 128 * (b2 + 1)])

            def attn_stage(c):
                # block-sparse attention for chunk c (past K/V only: sliding
                # window); leftover Wo groups of chunk c-1 fill the rope tail.
                with tc.tile_pool(name="scps", bufs=2, space="PSUM") as psc, \
                     tc.tile_pool(name="pvps", bufs=3, space="PSUM") as ppv, \
                     tc.tile_pool(name="denps", bufs=1, space="PSUM") as pdn:
                    kbs = list(range(max(0, 4 * c - 8), 4 * c + 4))
                    first_kb, last_kb = kbs[0], kbs[-1]
                    while emit_wo_group():
                        pass
                    atbs = []
                    for h0 in range(0, NQH, 2):
                        # process a PAIR of heads per key-block sweep: two
                        # independent score/exp chains per step keep ACT fed.
                        pvs = [ppv.tile([128, 512], F32, tag="pv", name="pv")
                               for _ in range(2)]
                        # one PSUM bank holds both heads' denominator rows
                        # (matmul out base partition must be 0/32/64)
                        pdnt = pdn.tile([33, 512], F32, tag="den", name="den")
                        dens = [pdnt[0:1, :], pdnt[32:33, :]]

                        def emit_pv(kb, pts):
                            # accumulate P@V and row-sums over exact visible slices.
                            sl = kb - 4 * c + 8
                            lo, hi = _slot_region(sl)
                            vsl = vnat[:, 128 * kb:128 * (kb + 1)]
                            for i in range(2):
                                nc.tensor.matmul(pvs[i][:, lo:hi], vsl, pts[i][:, lo:hi],
                                                 start=(kb == first_kb), stop=(kb == last_kb),
                                                 skip_group_check=True)
                                nc.tensor.matmul(dens[i][:, lo:hi], ones_t[:], pts[i][:, lo:hi],
                                                 start=(kb == first_kb), stop=(kb == last_kb),
                                                 skip_group_check=True)

                        pending = []
                        for kb in kbs:
                            sl = kb - 4 * c + 8
                            lo, hi = _slot_region(sl)
                            pts = []
                            for i in range(2):
                                sc = psc.tile([128, 512], F32, tag="sc", name="sc")
                                nc.tensor.matmul(sc[:, lo:hi], ktb[:, 128 * kb:128 * (kb + 1)],
                                                 qtb[h0 + i][:, SC * c + lo:SC * c + hi],
                                                 start=True, stop=True)
                                pt = ppt.tile([128, 512], BF16, tag="pt", name="pt")
                                nc.scalar.activation(pt[:, lo:hi], sc[:, lo:hi], EXP, scale=SCALE)
                                if sl <= 3:
                                    mofs = 128 * sl
                                    nc.vector.tensor_tensor(pt[:, mofs:mofs + 128],
                                                            pt[:, mofs:mofs + 128], mw_t[:], MULT)
                                elif sl >= 8:
                                    mofs = 128 * (sl - 8)
                                    nc.vector.tensor_tensor(pt[:, mofs:mofs + 128],
                                                            pt[:, mofs:mofs + 128], mc_t[:], MULT)
                                pts.append(pt)
                            pending.append((kb, pts))
                            if len(pending) > 1:
                                emit_pv(*pending.pop(0))
                        for pv_item in pending:
                            emit_pv(*pv_item)
                        for i in range(2):
                            # stage P@V out of PSUM via ACT so the bank frees
                            # immediately; normalize from SBUF off the
                            # critical path
                            pvu = pat.tile([128, 512], BF16, tag="pvu",
                                           name="pvu", bufs=2)
                            nc.scalar.copy(pvu[:], pvs[i][:])
                            dre = pmisc.tile([1, 512], BF16, tag="denr", name="denr")
                            with nc.allow_low_precision(reason="softmax denom to bf16"):
                                nc.vector.reciprocal(dre[:], dens[i])
                            dbc = pmisc.tile([128, 512], BF16, tag="denb", name="denb")
                            nc.gpsimd.partition_broadcast(dbc[:], dre[:])
                            at = pat.tile([128, 512], BF16, tag="atb", name="atb")
                            nc.vector.tensor_tensor(at[:], pvu[:], dbc[:], MULT)
                            atbs.append(at)
                return atbs

            for _it in range(niter):
                for c in range(NCH):
                    proj_stage(c)
                    atbs = attn_stage(c)
                    wo_state.update(c=c, atbs=atbs, m=0, obig=None)
            # the last chunk's Wo groups of each iteration ride the next
            # iteration's proj(0)/attn(0); only the final one drains here
            while emit_wo_group():
                pass


_NC_CACHE = {}


def _build(niter=1, fused=True):
    import os
    fused = os.environ.get("KERNEL_FUSED", "1" if fused else "0") == "1"
    key = (niter, fused)
    if key in _NC_CACHE:
        return _NC_CACHE[key]
    nc = bacc.Bacc("TRN2", target_bir_lowering=False, debug=False,
                   enable_asserts=True, num_devices=NCORES)
    dr = {}

    def din(name, shape, dt=F32):
        dr[name] = nc.dram_tensor(name, shape, dt, kind="ExternalInput").ap()

    din("ht", [HID, S], BF16)
    din("wqt", [HID, NQH * D], BF16)
    din("wkt", [HID, D], BF16)
    din("wvt", [HID, D], BF16)
    din("wot", [NQH * D, HID], BF16)
    din("ctab", [128, S])
    din("stab", [128, S])
    din("mcaus", [128, 128], BF16)
    din("mwin", [128, 128], BF16)
    out = nc.dram_tensor("out", [S, HID], BF16, kind="ExternalOutput").ap()

    with tile.TileContext(nc) as tc:
        _program(tc, dr, out, niter, fused)
    nc.compile()
    _NC_CACHE[key] = nc
    return nc


def make_in_maps(inputs):
    hs = np.asarray(inputs["hidden_states"], dtype=np.float32)
    Wq = np.asarray(inputs["Wq"], dtype=np.float32)
    Wk = np.asarray(inputs["Wk"], dtype=np.float32)
    Wv = np.asarray(inputs["Wv"], dtype=np.float32)
    Wo = np.asarray(inputs["Wo"], dtype=np.float32)
    pos = np.asarray(inputs["position_ids"]).reshape(-1)

    assert hs.shape == (1, S, HID), hs.shape
    H = hs[0]
    HT = np.ascontiguousarray(H.T)

    # RoPE tables in [d%64, s] layout (fp32, mirroring the reference math)
    inv = (1.0 / (ROPE_BASE ** (np.arange(0, D, 2, dtype=np.float32) / D))).astype(np.float32)
    ang = pos.astype(np.float32)[None, :] * inv[:, None]          # [64, S]
    cos64 = np.cos(ang).astype(np.float32)
    sin64 = np.sin(ang).astype(np.float32)
    ctab = np.concatenate([cos64, cos64], axis=0)                 # [128, S]
    stab = np.concatenate([-sin64, sin64], axis=0)                # sign-baked

    kk = np.arange(128)[:, None]
    qq = np.arange(128)[None, :]
    mcaus = (qq >= kk).astype(ml_dtypes.bfloat16)   # causal diag block, [k,q]
    mwin = (qq < kk).astype(ml_dtypes.bfloat16)     # window-edge block, [k,q]

    BF = ml_dtypes.bfloat16
    HTB = HT.astype(BF)
    in_maps = []
    for i in range(NCORES):
        in_maps.append({
            "ht": HTB,
            "wqt": np.ascontiguousarray(Wq[512 * i:512 * (i + 1), :].T).astype(BF),
            "wkt": np.ascontiguousarray(Wk[128 * i:128 * (i + 1), :].T).astype(BF),
            "wvt": np.ascontiguousarray(Wv[128 * i:128 * (i + 1), :].T).astype(BF),
            "wot": np.ascontiguousarray(Wo[:, 512 * i:512 * (i + 1)].T).astype(BF),
            "ctab": ctab,
            "stab": stab,
            "mcaus": mcaus,
            "mwin": mwin,
        })

    return in_maps


def kernel(**inputs):
    in_maps = make_in_maps(inputs)
    nc = _build()
    res = run_bass_kernel_spmd(nc, in_maps, core_ids=list(range(NCORES)))

    acc = np.zeros((S, HID), dtype=np.float32)
    for r in res.results:
        acc += r["out"].astype(np.float32)
    return acc.reshape(1, S, HID)



# revision 42
# speedup vs baseline: 1.4415x; 1.1487x over previous
"""Trainium2 Bass kernel: Mistral-style GQA attention with sliding-window mask.

Problem: hidden [1,2048,4096] -> Wq/Wk/Wv projections (32 q heads, 8 kv heads,
head_dim 128) -> RoPE -> sliding-window (1024) causal attention -> Wo.

Sharding: tensor-parallel over heads on 8 NeuronCores. Core i owns KV head i
and query heads 4i..4i+3 (Wq/Wk/Wv row-sharded, Wo column-sharded). Each core
computes partial_i = attn_heads_i @ Wo_i^T in HBM; host sums the 8 partials
(the TP all-reduce) to produce the full output.

On-device per core:
  phase A: stream H^T tiles from HBM, cast bf16, matmul into Q^T/K^T/V^T
           (layout [head_dim, seq]), fused RoPE on Q/K out of PSUM,
           V^T transposed to natural [seq, head_dim] via DMA-transpose.
  phase B: block-sparse attention per 512-query chunk: scores^T = K^T-block
           x Q^T-chunk on PE, exp on ACT (PSUM->SBUF bf16), static triangular
           masks on DVE, P@V and row-sum denominators on PE (ones-vector
           matmul), reciprocal+partition_broadcast for normalization, then
           Wo matmuls and fp32 output copy.
"""

import sys

for _p in ("/opt/trn_rl_repo", "/root/.axon_site/_ro/trn_rl_repo"):
    if _p not in sys.path:
        sys.path.insert(0, _p)

import numpy as np
import ml_dtypes

import concourse.bass as bass  # noqa: F401  (registers engine classes)
import concourse.mybir as mybir
import concourse.tile as tile
from concourse import bacc
from concourse.bass_utils import run_bass_kernel_spmd

S = 2048
HID = 4096
D = 128
NQH = 4          # query heads per core
NCORES = 8
SC = 512         # seq chunk
NCH = S // SC
KT = HID // 128  # contraction tiles
WINDOW = 1024
ROPE_BASE = 10000.0
SCALE = 1.0 / float(np.sqrt(D))

F32 = mybir.dt.float32
BF16 = mybir.dt.bfloat16
MULT = mybir.AluOpType.mult
ADD = mybir.AluOpType.add
SUB = mybir.AluOpType.subtract
EXP = mybir.ActivationFunctionType.Exp

# ptb slot layout: slot sl = kb - 4c + 8 for key-block kb in chunk c.
# exp-written region per slot, and statically-zero (memset once) regions.
def _slot_region(sl):
    lo = 128 * (sl - 8) if sl >= 8 else 0
    hi = 512 if sl >= 3 else 128 * (sl + 1)
    return lo, hi

_INVIS = []
for _sl in range(12):
    _lo, _hi = _slot_region(_sl)
    if _lo > 0:
        _INVIS.append((_sl, 0, _lo))
    if _hi < 512:
        _INVIS.append((_sl, _hi, 512))


def _program(tc, dr, out, niter=1, fused=True):
    nc = tc.nc
    ht, wqt, wkt, wvt, wot = dr["ht"], dr["wqt"], dr["wkt"], dr["wvt"], dr["wot"]
    ctab, stab, mcaus, mwin = dr["ctab"], dr["stab"], dr["mcaus"], dr["mwin"]

    def _copy(eng, out_ap, in_ap):
        if eng is nc.scalar:
            eng.copy(out_ap, in_ap)
        else:
            eng.tensor_copy(out_ap, in_ap)

    # Wo PSUM->SBUF copies: DVE mid-chunk (it is idle there), ACT for the
    # boundary groups (DVE ropes then; a Wo copy queued behind rope ops
    # holds its PSUM bank and stalls PE's next Wo group)
    def pcopy_rr(out_ap, in_ap, eng=None):
        _copy(eng or nc.scalar, out_ap, in_ap)

    from contextlib import ExitStack
    if True:
        with ExitStack() as ctx:
            pw = ctx.enter_context(tc.tile_pool(name="persist", bufs=1))
            prt = ctx.enter_context(tc.tile_pool(name="ropet", bufs=2))

            wqb = pw.tile([128, KT * 512], BF16, name="wqb")
            wkb = pw.tile([128, KT * 128], BF16, name="wkb")
            wvb = pw.tile([128, KT * 128], BF16, name="wvb")
            wob = pw.tile([128, NQH * HID], BF16, name="wob")
            qtb = [pw.tile([128, S], BF16, name=f"qtb{h}") for h in range(NQH)]
            ktb = pw.tile([128, S], BF16, name="ktb")
            vtb = pw.tile([128, S], BF16, name="vtb")
            vnat = pw.tile([128, S], BF16, name="vnat")
            cs_t = pw.tile([128, S], F32, name="cs_t")
            sn_t = pw.tile([128, S], F32, name="sn_t")
            mc_t = pw.tile([128, 128], BF16, name="mc_t")
            mw_t = pw.tile([128, 128], BF16, name="mw_t")
            ones_t = pw.tile([128, 1], BF16, name="ones_t")
            # q2/q3 staging, pre-rotated: [pre(512), rot(512)] per head
            psq = pw.tile([128, 2048], BF16, name="psq")

            nc.gpsimd.memset(ones_t[:], 1.0)

            def _rope_staged(dst, pre, rot, c, co, w):
                """staged rope: pre/rot already in SBUF (rot = half-swapped),
                full-width same-base-partition ops (SBUF+SBUF DVE operands
                must share a base partition)."""
                lo, hi = SC * c + co, SC * c + co + w
                t1 = prt.tile([128, w], F32, tag="rt1", name="st1")
                t2 = prt.tile([128, w], F32, tag="rt2", name="st2")
                nc.vector.tensor_tensor(t1[:], pre, cs_t[:, lo:hi], MULT)
                nc.vector.tensor_tensor(t2[:], rot, sn_t[:, lo:hi], MULT)
                nc.vector.tensor_tensor(dst, t1[:], t2[:], ADD)

            def _rope(dst, p, c, co, w, eng="dve"):
                """dst[bf16 [128,w] slice] = rope(p [[128,w] slice, PSUM or SBUF]),
                chunk c col-offset co.

                cs_t is cos duplicated across both halves; sn_t is sign-baked
                sin: rows 0:64 = -sin, rows 64:128 = +sin, so
                out = q*cos + rot(q)*sn with rot a plain half-swap.
                """
                use_gp = eng == "gp"
                lo, hi = SC * c + co, SC * c + co + w
                csl = cs_t[:, lo:hi]
                snl = sn_t[:, lo:hi]
                if use_gp:
                    pre = prt.tile([128, w], BF16, tag="rpre", name="rpre", bufs=2)
                    rot = prt.tile([128, w], BF16, tag="rrot", name="rrot", bufs=2)
                    nc.scalar.copy(pre[:], p[:])
                    nc.scalar.copy(rot[0:64, :], p[64:128, :])
                    nc.scalar.copy(rot[64:128, :], p[0:64, :])
                    g1 = prt.tile([128, w], F32, tag="rt1", name="g1")
                    g2 = prt.tile([128, w], F32, tag="rt2", name="g2")
                    nc.gpsimd.tensor_tensor(g1[:], pre[:], csl, MULT)
                    nc.gpsimd.tensor_tensor(g2[:], rot[:], snl, MULT)
                    nc.gpsimd.tensor_tensor(dst[:, :], g1[:], g2[:], ADD)
                    return
                e = nc.vector
                t1 = prt.tile([64, w], F32, tag="rt1", name="rt1")
                t2 = prt.tile([64, w], F32, tag="rt2", name="rt2")
                e.tensor_tensor(t1[:], p[0:64, :], csl[0:64, :], MULT)
                e.tensor_tensor(t2[:], p[64:128, :], snl[0:64, :], MULT)
                e.tensor_tensor(dst[0:64, :], t1[:], t2[:], ADD)
                t3 = prt.tile([64, w], F32, tag="rt1", name="rt3")
                t4 = prt.tile([64, w], F32, tag="rt2", name="rt4")
                e.tensor_tensor(t3[:], p[64:128, :], csl[64:128, :], MULT)
                e.tensor_tensor(t4[:], p[0:64, :], snl[64:128, :], MULT)
                e.tensor_tensor(dst[64:128, :], t3[:], t4[:], ADD)

            phb = ctx.enter_context(tc.tile_pool(name="htbp", bufs=12))
            ppt = ctx.enter_context(tc.tile_pool(name="ptp", bufs=8))
            pmisc = ctx.enter_context(tc.tile_pool(name="miscb", bufs=2))
            pat = ctx.enter_context(tc.tile_pool(name="atbp", bufs=8))
            posb = ctx.enter_context(tc.tile_pool(name="osbp", bufs=2))
            # Wo-output PSUM pool stays open across proj+attn of every chunk:
            # 2 banks here + 6 proj banks = 8; 2 + 6 attn banks = 8.
            pop = ctx.enter_context(tc.tile_pool(name="outps", bufs=2, space="PSUM"))

            # Wo emission: one group = one [128q, 512hid] output tile of the
            # PREVIOUS chunk; groups interleave into the proj k-loop (PSUM
            # budget) and the attn pre-loop (covers the rope-tail window).
            wo_state = {"c": None, "atbs": None, "m": 0, "obig": None}

            def emit_wo_group():
                st = wo_state
                if st["c"] is None or st["m"] >= 32:
                    return False
                wj, wn = st["m"] // 8, st["m"] % 8
                wc, watbs = st["c"], st["atbs"]
                if wn % 4 == 0:
                    st["obig"] = posb.tile([128, HID // 2], BF16, tag="osb", name="osb")
                po = pop.tile([128, 512], F32, tag="po", name="po")
                for h in range(NQH):
                    nc.tensor.matmul(po[:], watbs[h][:, 128 * wj:128 * (wj + 1)],
                                     wob[:, HID * h + 512 * wn:HID * h + 512 * (wn + 1)],
                                     start=(h == 0), stop=(h == NQH - 1))
                pcopy_rr(st["obig"][:, 512 * (wn % 4):512 * (wn % 4 + 1)], po[:],
                         eng=(nc.vector if st["m"] < 24 else nc.scalar))
                if wn % 4 == 3:
                    nc.scalar.dma_start(
                        out[SC * wc + 128 * wj:SC * wc + 128 * (wj + 1),
                            2048 * (wn // 4):2048 * (wn // 4 + 1)],
                        st["obig"][:])
                st["m"] += 1
                return True

            def proj_stage(c):
                # projections for chunk c, in two 256-col seq halves: rope of
                # half 0 runs on DVE/GP/ACT underneath PE's half-1 k-loop, so
                # only the half-1 rope tail is exposed at the chunk boundary
                # (the attn PSUM pools can only open once the proj pool closes,
                # i.e. after the last rope drains its bank). Chunk 0 stays
                # full-width — its k-loop must cover the serial weight-DMA
                # stream — and stages q2/q3 through SBUF so the pool close
                # only waits on the K/q0/q1 ropes.
                halves = 1 if c == 0 else 2
                w = SC // halves
                with tc.tile_pool(name="projps", bufs=6, space="PSUM") as ppp:
                    ps6 = [ppp.tile([128, 512], F32, tag="proj", name=f"proj{c}_{i}")
                           for i in range(6)]
                    it = 0
                    for half in range(halves):
                        co = w * half
                        hb = None
                        for k in range(KT):
                            g = k // 4
                            if k % 4 == 0:
                                hb = phb.tile([128, 4 * w], BF16,
                                              tag=f"htb{halves}", name="hb",
                                              bufs=(4 if halves == 1 else 12))
                                nc.sync.dma_start(
                                    hb[:].rearrange("p (k j) -> p k j", j=w),
                                    ht[512 * g:512 * (g + 1),
                                       SC * c + co:SC * c + co + w].rearrange(
                                        "(k p) j -> p k j", p=128))
                                if c == 0 and g < 4:
                                    nc.sync.dma_start(
                                        wqb[:, 4096 * g:4096 * (g + 1)].rearrange(
                                            "p (k j) -> p k j", j=512),
                                        wqt[1024 * g:1024 * (g + 1), :].rearrange(
                                            "(k p) j -> p k j", p=128))
                                    nc.sync.dma_start(
                                        wkb[:, 1024 * g:1024 * (g + 1)].rearrange(
                                            "p (k j) -> p k j", j=128),
                                        wkt[1024 * g:1024 * (g + 1), :].rearrange(
                                            "(k p) j -> p k j", p=128))
                                    nc.sync.dma_start(
                                        wvb[:, 1024 * g:1024 * (g + 1)].rearrange(
                                            "p (k j) -> p k j", j=128),
                                        wvt[1024 * g:1024 * (g + 1), :].rearrange(
                                            "(k p) j -> p k j", p=128))
                                if c == 0 and g == 2:
                                    nc.sync.dma_start(cs_t[:, 0:SC], ctab[:, 0:SC])
                                    nc.sync.dma_start(sn_t[:, 0:SC], stab[:, 0:SC])
                                    nc.sync.dma_start(mc_t[:], mcaus[:])
                                    nc.sync.dma_start(mw_t[:], mwin[:])
                                if c == 0 and g >= 4:
                                    # wob halves ride the c0 tail (the weight
                                    # stream has drained by k=16)
                                    wh = 2 * (g - 4)
                                    for j in (wh, wh + 1):
                                        nc.sync.dma_start(
                                            wob[:, 2048 * j:2048 * (j + 1)],
                                            wot[128 * (j // 2):128 * (j // 2 + 1),
                                                4096 * (j % 2) // 2:
                                                4096 * (j % 2) // 2 + 2048])
                                if c == 1 and half == 0 and g == 0:
                                    nc.sync.dma_start(cs_t[:, SC:], ctab[:, SC:])
                                    nc.sync.dma_start(sn_t[:, SC:], stab[:, SC:])
                            hsl = hb[:, w * (k % 4):w * (k % 4 + 1)]
                            first, last = k == 0, k == KT - 1
                            for h in range(NQH):
                                nc.tensor.matmul(
                                    ps6[h][:, co:co + w],
                                    wqb[:, 512 * k + 128 * h:512 * k + 128 * (h + 1)],
                                    hsl, start=first, stop=last, skip_group_check=True)
                            nc.tensor.matmul(ps6[4][:, co:co + w],
                                             wkb[:, 128 * k:128 * (k + 1)], hsl,
                                             start=first, stop=last, skip_group_check=True)
                            nc.tensor.matmul(ps6[5][:, co:co + w],
                                             wvb[:, 128 * k:128 * (k + 1)], hsl,
                                             start=first, stop=last, skip_group_check=True)
                            if it >= 8 and wo_state["m"] < 24:
                                emit_wo_group()
                            it += 1
                        if half == halves - 1:
                            # their ACT copies precede the rope pre-copies in
                            # ACT's in-order queue, keeping the po-bank ring
                            # turning while DVE ropes
                            emit_wo_group()
                            emit_wo_group()
                        for r0 in range(0, w, 256):
                            # q1 before K on DVE: the attn score pool reuses
                            # the q0/q1 PSUM banks, so drain those first (q0
                            # frees via its ACT pre-copies)
                            _rope(qtb[0][:, SC * c + co + r0:SC * c + co + r0 + 256],
                                  ps6[0][:, co + r0:co + r0 + 256], c, co + r0, 256,
                                  eng="gp")
                            _rope(qtb[1][:, SC * c + co + r0:SC * c + co + r0 + 256],
                                  ps6[1][:, co + r0:co + r0 + 256], c, co + r0, 256,
                                  eng="dve")
                            _rope(ktb[:, SC * c + co + r0:SC * c + co + r0 + 256],
                                  ps6[4][:, co + r0:co + r0 + 256], c, co + r0, 256,
                                  eng="dve")
                        if halves == 1:
                            # free the q2/q3 banks early: stage to SBUF via
                            # ACT (rot half pre-swapped), rope later (pair 1
                            # needs them ~10us on)
                            for qi in (2, 3):
                                b = 1024 * (qi - 2)
                                nc.scalar.copy(psq[:, b:b + 512], ps6[qi][:])
                                nc.scalar.copy(psq[0:64, b + 512:b + 1024],
                                               ps6[qi][64:128, :])
                                nc.scalar.copy(psq[64:128, b + 512:b + 1024],
                                               ps6[qi][0:64, :])
                        else:
                            _rope(qtb[2][:, SC * c + co:SC * c + co + w],
                                  ps6[2][:, co:co + w], c, co, w, eng="dve")
                            _rope(qtb[3][:, SC * c + co:SC * c + co + w],
                                  ps6[3][:, co:co + w], c, co, w, eng="dve")
                        nc.scalar.copy(vtb[:, SC * c + co:SC * c + co + w],
                                       ps6[5][:, co:co + w])
                        if halves == 1:
                            for r0 in range(0, SC, 256):
                                _rope_staged(
                                    qtb[2][:, SC * c + r0:SC * c + r0 + 256],
                                    psq[:, r0:r0 + 256],
                                    psq[:, 512 + r0:512 + r0 + 256], c, r0, 256)
                                _rope_staged(
                                    qtb[3][:, SC * c + r0:SC * c + r0 + 256],
                                    psq[:, 1024 + r0:1024 + r0 + 256],
                                    psq[:, 1536 + r0:1536 + r0 + 256], c, r0, 256)
                        for b2 in range(w // 128):
                            bo = 128 * ((SC // 128) * c + (w // 128) * half + b2)
                            nc.scalar.dma_start_transpose(
                                vnat[:, bo:bo + 128],
                                vtb[:, SC * c + co + 128 * b2:SC * c + co + 128 * (b2 + 1)])

            def attn_stage(c):
                # block-sparse attention for chunk c (past K/V only: sliding
                # window); leftover Wo groups of chunk c-1 fill the rope tail.
                with tc.tile_pool(name="scps", bufs=3, space="PSUM") as psc, \
                     tc.tile_pool(name="pvps", bufs=2, space="PSUM") as ppv, \
                     tc.tile_pool(name="denps", bufs=1, space="PSUM") as pdn:
                    kbs = list(range(max(0, 4 * c - 8), 4 * c + 4))
                    first_kb, last_kb = kbs[0], kbs[-1]
                    while emit_wo_group():
                        pass
                    atbs = []
                    for h0 in range(0, NQH, 2):
                        # process a PAIR of heads per key-block sweep: two
                        # independent score/exp chains per step keep ACT fed.
                        pvs = [ppv.tile([128, 512], F32, tag="pv", name="pv")
                               for _ in range(2)]
                        # one PSUM bank holds both heads' denominator rows
                        # (matmul out base partition must be 0/32/64)
                        pdnt = pdn.tile([33, 512], F32, tag="den", name="den")
                        dens = [pdnt[0:1, :], pdnt[32:33, :]]

                        def emit_pv(kb, pts):
                            # accumulate P@V and row-sums over exact visible slices.
                            sl = kb - 4 * c + 8
                            lo, hi = _slot_region(sl)
                            vsl = vnat[:, 128 * kb:128 * (kb + 1)]
                            for i in range(2):
                                nc.tensor.matmul(pvs[i][:, lo:hi], vsl, pts[i][:, lo:hi],
                                                 start=(kb == first_kb), stop=(kb == last_kb),
                                                 skip_group_check=True)
                                nc.tensor.matmul(dens[i][:, lo:hi], ones_t[:], pts[i][:, lo:hi],
                                                 start=(kb == first_kb), stop=(kb == last_kb),
                                                 skip_group_check=True)

                        pending = []
                        for kb in kbs:
                            sl = kb - 4 * c + 8
                            lo, hi = _slot_region(sl)
                            pts = []
                            for i in range(2):
                                sc = psc.tile([128, 512], F32, tag="sc", name="sc")
                                nc.tensor.matmul(sc[:, lo:hi], ktb[:, 128 * kb:128 * (kb + 1)],
                                                 qtb[h0 + i][:, SC * c + lo:SC * c + hi],
                                                 start=True, stop=True)
                                pt = ppt.tile([128, 512], BF16, tag="pt", name="pt")
                                nc.scalar.activation(pt[:, lo:hi], sc[:, lo:hi], EXP, scale=SCALE)
                                if sl <= 3:
                                    mofs = 128 * sl
                                    nc.vector.tensor_tensor(pt[:, mofs:mofs + 128],
                                                            pt[:, mofs:mofs + 128], mw_t[:], MULT)
                                elif sl >= 8:
                                    mofs = 128 * (sl - 8)
                                    nc.vector.tensor_tensor(pt[:, mofs:mofs + 128],
                                                            pt[:, mofs:mofs + 128], mc_t[:], MULT)
                                pts.append(pt)
                            pending.append((kb, pts))
                            if len(pending) > 2:
                                emit_pv(*pending.pop(0))
                        for pv_item in pending:
                            emit_pv(*pv_item)
                        for i in range(2):
                            # stage P@V out of PSUM via ACT so the bank frees
                            # immediately; normalize from SBUF off the
                            # critical path
                            pvu = pat.tile([128, 512], BF16, tag="pvu",
                                           name="pvu", bufs=2)
                            nc.scalar.copy(pvu[:], pvs[i][:])
                            dre = pmisc.tile([1, 512], BF16, tag="denr", name="denr")
                            with nc.allow_low_precision(reason="softmax denom to bf16"):
                                nc.vector.reciprocal(dre[:], dens[i])
                            dbc = pmisc.tile([128, 512], BF16, tag="denb", name="denb")
                            nc.gpsimd.partition_broadcast(dbc[:], dre[:])
                            at = pat.tile([128, 512], BF16, tag="atb", name="atb")
                            nc.vector.tensor_tensor(at[:], pvu[:], dbc[:], MULT)
                            atbs.append(at)
                return atbs

            for _it in range(niter):
                for c in range(NCH):
                    proj_stage(c)
                    atbs = attn_stage(c)
                    wo_state.update(c=c, atbs=atbs, m=0, obig=None)
            # the last chunk's Wo groups of each iteration ride the next
            # iteration's proj(0)/attn(0); only the final one drains here
            while emit_wo_group():
                pass


_NC_CACHE = {}


def _build(niter=1, fused=True):
    import os
    fused = os.environ.get("KERNEL_FUSED", "1" if fused else "0") == "1"
    key = (niter, fused)
    if key in _NC_CACHE:
        return _NC_CACHE[key]
    nc = bacc.Bacc("TRN2", target_bir_lowering=False, debug=False,
                   enable_asserts=True, num_devices=NCORES)
    dr = {}

    def din(name, shape, dt=F32):
        dr[name] = nc.dram_tensor(name, shape, dt, kind="ExternalInput").ap()

    din("ht", [HID, S], BF16)
    din("wqt", [HID, NQH * D], BF16)
    din("wkt", [HID, D], BF16)
    din("wvt", [HID, D], BF16)
    din("wot", [NQH * D, HID], BF16)
    din("ctab", [128, S])
    din("stab", [128, S])
    din("mcaus", [128, 128], BF16)
    din("mwin", [128, 128], BF16)
    out = nc.dram_tensor("out", [S, HID], BF16, kind="ExternalOutput").ap()

    with tile.TileContext(nc) as tc:
        _program(tc, dr, out, niter, fused)
    nc.compile()
    _NC_CACHE[key] = nc
    return nc


def make_in_maps(inputs):
    hs = np.asarray(inputs["hidden_states"], dtype=np.float32)
    Wq = np.asarray(inputs["Wq"], dtype=np.float32)
    Wk = np.asarray(inputs["Wk"], dtype=np.float32)
    Wv = np.asarray(inputs["Wv"], dtype=np.float32)
    Wo = np.asarray(inputs["Wo"], dtype=np.float32)
    pos = np.asarray(inputs["position_ids"]).reshape(-1)

    assert hs.shape == (1, S, HID), hs.shape
    H = hs[0]
    HT = np.ascontiguousarray(H.T)

    # RoPE tables in [d%64, s] layout (fp32, mirroring the reference math)
    inv = (1.0 / (ROPE_BASE ** (np.arange(0, D, 2, dtype=np.float32) / D))).astype(np.float32)
    ang = pos.astype(np.float32)[None, :] * inv[:, None]          # [64, S]
    cos64 = np.cos(ang).astype(np.float32)
    sin64 = np.sin(ang).astype(np.float32)
    ctab = np.concatenate([cos64, cos64], axis=0)                 # [128, S]
    stab = np.concatenate([-sin64, sin64], axis=0)                # sign-baked

    kk = np.arange(128)[:, None]
    qq = np.arange(128)[None, :]
    mcaus = (qq >= kk).astype(ml_dtypes.bfloat16)   # causal diag block, [k,q]
    mwin = (qq < kk).astype(ml_dtypes.bfloat16)     # window-edge block, [k,q]

    BF = ml_dtypes.bfloat16
    HTB = HT.astype(BF)
    in_maps = []
    for i in range(NCORES):
        in_maps.append({
            "ht": HTB,
            "wqt": np.ascontiguousarray(Wq[512 * i:512 * (i + 1), :].T).astype(BF),
            "wkt": np.ascontiguousarray(Wk[128 * i:128 * (i + 1), :].T).astype(BF),
            "wvt": np.ascontiguousarray(Wv[128 * i:128 * (i + 1), :].T).astype(BF),
            "wot": np.ascontiguousarray(Wo[:, 512 * i:512 * (i + 1)].T).astype(BF),
            "ctab": ctab,
            "stab": stab,
            "mcaus": mcaus,
            "mwin": mwin,
        })

    return in_maps


def kernel(**inputs):
    in_maps = make_in_maps(inputs)
    nc = _build()
    res = run_bass_kernel_spmd(nc, in_maps, core_ids=list(range(NCORES)))

    acc = np.zeros((S, HID), dtype=np.float32)
    for r in res.results:
        acc += r["out"].astype(np.float32)
    return acc.reshape(1, S, HID)



# revision 45
# speedup vs baseline: 1.5664x; 1.0866x over previous
"""Trainium2 Bass kernel: Mistral-style GQA attention with sliding-window mask.

Problem: hidden [1,2048,4096] -> Wq/Wk/Wv projections (32 q heads, 8 kv heads,
head_dim 128) -> RoPE -> sliding-window (1024) causal attention -> Wo.

Sharding: tensor-parallel over heads on 8 NeuronCores. Core i owns KV head i
and query heads 4i..4i+3 (Wq/Wk/Wv row-sharded, Wo column-sharded). Each core
computes partial_i = attn_heads_i @ Wo_i^T in HBM; host sums the 8 partials
(the TP all-reduce) to produce the full output.

On-device per core:
  phase A: stream H^T tiles from HBM, cast bf16, matmul into Q^T/K^T/V^T
           (layout [head_dim, seq]), fused RoPE on Q/K out of PSUM,
           V^T transposed to natural [seq, head_dim] via DMA-transpose.
  phase B: block-sparse attention per 512-query chunk: scores^T = K^T-block
           x Q^T-chunk on PE, exp on ACT (PSUM->SBUF bf16), static triangular
           masks on DVE, P@V and row-sum denominators on PE (ones-vector
           matmul), reciprocal+partition_broadcast for normalization, then
           Wo matmuls and fp32 output copy.
"""

import sys

for _p in ("/opt/trn_rl_repo", "/root/.axon_site/_ro/trn_rl_repo"):
    if _p not in sys.path:
        sys.path.insert(0, _p)

import numpy as np
import ml_dtypes

import concourse.bass as bass  # noqa: F401  (registers engine classes)
import concourse.mybir as mybir
import concourse.tile as tile
from concourse import bacc
from concourse.bass_utils import run_bass_kernel_spmd

S = 2048
HID = 4096
D = 128
NQH = 4          # query heads per core
NCORES = 8
SC = 512         # seq chunk
NCH = S // SC
KT = HID // 128  # contraction tiles
WINDOW = 1024
ROPE_BASE = 10000.0
SCALE = 1.0 / float(np.sqrt(D))

F32 = mybir.dt.float32
BF16 = mybir.dt.bfloat16
MULT = mybir.AluOpType.mult
ADD = mybir.AluOpType.add
SUB = mybir.AluOpType.subtract
EXP = mybir.ActivationFunctionType.Exp

# ptb slot layout: slot sl = kb - 4c + 8 for key-block kb in chunk c.
# exp-written region per slot, and statically-zero (memset once) regions.
def _slot_region(sl):
    lo = 128 * (sl - 8) if sl >= 8 else 0
    hi = 512 if sl >= 3 else 128 * (sl + 1)
    return lo, hi

_INVIS = []
for _sl in range(12):
    _lo, _hi = _slot_region(_sl)
    if _lo > 0:
        _INVIS.append((_sl, 0, _lo))
    if _hi < 512:
        _INVIS.append((_sl, _hi, 512))


def _program(tc, dr, out, niter=1, fused=True):
    nc = tc.nc
    ht, wqt, wkt, wvt, wot = dr["ht"], dr["wqt"], dr["wkt"], dr["wvt"], dr["wot"]
    ctab, stab, mcaus, mwin = dr["ctab"], dr["stab"], dr["mcaus"], dr["mwin"]

    def _copy(eng, out_ap, in_ap):
        if eng is nc.scalar:
            eng.copy(out_ap, in_ap)
        else:
            eng.tensor_copy(out_ap, in_ap)

    # Wo PSUM->SBUF copies: DVE mid-chunk (it is idle there), ACT for the
    # boundary groups (DVE ropes then; a Wo copy queued behind rope ops
    # holds its PSUM bank and stalls PE's next Wo group)
    def pcopy_rr(out_ap, in_ap, eng=None):
        _copy(eng or nc.scalar, out_ap, in_ap)

    from contextlib import ExitStack
    if True:
        with ExitStack() as ctx:
            pw = ctx.enter_context(tc.tile_pool(name="persist", bufs=1))
            prt = ctx.enter_context(tc.tile_pool(name="ropet", bufs=2))

            wqb = pw.tile([128, KT * 512], BF16, name="wqb")
            wkb = pw.tile([128, KT * 128], BF16, name="wkb")
            wvb = pw.tile([128, KT * 128], BF16, name="wvb")
            wob = pw.tile([128, NQH * HID], BF16, name="wob")
            qtb = [pw.tile([128, S], BF16, name=f"qtb{h}") for h in range(NQH)]
            ktb = pw.tile([128, S], BF16, name="ktb")
            vtb = pw.tile([128, S], BF16, name="vtb")
            vnat = pw.tile([128, S], BF16, name="vnat")
            cs_t = pw.tile([128, S], F32, name="cs_t")
            sn_t = pw.tile([128, S], F32, name="sn_t")
            mc_t = pw.tile([128, 128], BF16, name="mc_t")
            mw_t = pw.tile([128, 128], BF16, name="mw_t")
            ones_t = pw.tile([128, 1], BF16, name="ones_t")
            # q2/q3 staging, pre-rotated: [pre(512), rot(512)] per head
            psq = pw.tile([128, 2048], BF16, name="psq")

            nc.gpsimd.memset(ones_t[:], 1.0)

            def _rope_staged(dst, pre, rot, c, co, w):
                """staged rope: pre/rot already in SBUF (rot = half-swapped),
                full-width same-base-partition ops (SBUF+SBUF DVE operands
                must share a base partition)."""
                lo, hi = SC * c + co, SC * c + co + w
                t1 = prt.tile([128, w], F32, tag="rt1", name="st1")
                t2 = prt.tile([128, w], F32, tag="rt2", name="st2")
                nc.vector.tensor_tensor(t1[:], pre, cs_t[:, lo:hi], MULT)
                nc.vector.tensor_tensor(t2[:], rot, sn_t[:, lo:hi], MULT)
                nc.vector.tensor_tensor(dst, t1[:], t2[:], ADD)

            def _rope(dst, p, c, co, w, eng="dve"):
                """dst[bf16 [128,w] slice] = rope(p [[128,w] slice, PSUM or SBUF]),
                chunk c col-offset co.

                cs_t is cos duplicated across both halves; sn_t is sign-baked
                sin: rows 0:64 = -sin, rows 64:128 = +sin, so
                out = q*cos + rot(q)*sn with rot a plain half-swap.
                """
                use_gp = eng == "gp"
                lo, hi = SC * c + co, SC * c + co + w
                csl = cs_t[:, lo:hi]
                snl = sn_t[:, lo:hi]
                if use_gp:
                    pre = prt.tile([128, w], BF16, tag="rpre", name="rpre", bufs=2)
                    rot = prt.tile([128, w], BF16, tag="rrot", name="rrot", bufs=2)
                    nc.scalar.copy(pre[:], p[:])
                    nc.scalar.copy(rot[0:64, :], p[64:128, :])
                    nc.scalar.copy(rot[64:128, :], p[0:64, :])
                    g1 = prt.tile([128, w], F32, tag="rt1", name="g1")
                    g2 = prt.tile([128, w], F32, tag="rt2", name="g2")
                    nc.gpsimd.tensor_tensor(g1[:], pre[:], csl, MULT)
                    nc.gpsimd.tensor_tensor(g2[:], rot[:], snl, MULT)
                    nc.gpsimd.tensor_tensor(dst[:, :], g1[:], g2[:], ADD)
                    return
                e = nc.vector
                t1 = prt.tile([64, w], F32, tag="rt1", name="rt1")
                t2 = prt.tile([64, w], F32, tag="rt2", name="rt2")
                e.tensor_tensor(t1[:], p[0:64, :], csl[0:64, :], MULT)
                e.tensor_tensor(t2[:], p[64:128, :], snl[0:64, :], MULT)
                e.tensor_tensor(dst[0:64, :], t1[:], t2[:], ADD)
                t3 = prt.tile([64, w], F32, tag="rt1", name="rt3")
                t4 = prt.tile([64, w], F32, tag="rt2", name="rt4")
                e.tensor_tensor(t3[:], p[64:128, :], csl[64:128, :], MULT)
                e.tensor_tensor(t4[:], p[0:64, :], snl[64:128, :], MULT)
                e.tensor_tensor(dst[64:128, :], t3[:], t4[:], ADD)

            phb = ctx.enter_context(tc.tile_pool(name="htbp", bufs=12))
            ppt = ctx.enter_context(tc.tile_pool(name="ptp", bufs=8))
            pmisc = ctx.enter_context(tc.tile_pool(name="miscb", bufs=2))
            pat = ctx.enter_context(tc.tile_pool(name="atbp", bufs=8))
            posb = ctx.enter_context(tc.tile_pool(name="osbp", bufs=2))
            # Wo-output PSUM pool stays open across proj+attn of every chunk:
            # 2 banks here + 6 proj banks = 8; 2 + 6 attn banks = 8.
            pop = ctx.enter_context(tc.tile_pool(name="outps", bufs=2, space="PSUM"))

            # Wo emission: one group = one [128q, 512hid] output tile of the
            # PREVIOUS chunk; groups interleave into the proj k-loop (PSUM
            # budget) and the attn pre-loop (covers the rope-tail window).
            wo_state = {"c": None, "atbs": None, "m": 0, "obig": None}

            def emit_wo_group():
                st = wo_state
                if st["c"] is None or st["m"] >= 32:
                    return False
                wj, wn = st["m"] // 8, st["m"] % 8
                wc, watbs = st["c"], st["atbs"]
                if wn % 4 == 0:
                    st["obig"] = posb.tile([128, HID // 2], BF16, tag="osb", name="osb")
                po = pop.tile([128, 512], F32, tag="po", name="po")
                for h in range(NQH):
                    nc.tensor.matmul(po[:], watbs[h][:, 128 * wj:128 * (wj + 1)],
                                     wob[:, HID * h + 512 * wn:HID * h + 512 * (wn + 1)],
                                     start=(h == 0), stop=(h == NQH - 1))
                pcopy_rr(st["obig"][:, 512 * (wn % 4):512 * (wn % 4 + 1)], po[:],
                         eng=(nc.vector if st["m"] < 24 else nc.scalar))
                if wn % 4 == 3:
                    nc.scalar.dma_start(
                        out[SC * wc + 128 * wj:SC * wc + 128 * (wj + 1),
                            2048 * (wn // 4):2048 * (wn // 4 + 1)],
                        st["obig"][:])
                st["m"] += 1
                return True

            def proj_stage(c):
                # projections for chunk c, in two 256-col seq halves: rope of
                # half 0 runs on DVE/GP/ACT underneath PE's half-1 k-loop, so
                # only the half-1 rope tail is exposed at the chunk boundary
                # (the attn PSUM pools can only open once the proj pool closes,
                # i.e. after the last rope drains its bank). Chunk 0 stays
                # full-width — its k-loop must cover the serial weight-DMA
                # stream — and stages q2/q3 through SBUF so the pool close
                # only waits on the K/q0/q1 ropes.
                halves = 2
                w = SC // halves
                with tc.tile_pool(name="projps", bufs=6, space="PSUM") as ppp:
                    ps6 = [ppp.tile([128, 512], F32, tag="proj", name=f"proj{c}_{i}")
                           for i in range(6)]
                    it = 0
                    for half in range(halves):
                        co = w * half
                        hb = None
                        for k in range(KT):
                            g = k // 4
                            if k % 4 == 0:
                                hb = phb.tile([128, 4 * w], BF16,
                                              tag="htb", name="hb", bufs=12)
                                nc.sync.dma_start(
                                    hb[:].rearrange("p (k j) -> p k j", j=w),
                                    ht[512 * g:512 * (g + 1),
                                       SC * c + co:SC * c + co + w].rearrange(
                                        "(k p) j -> p k j", p=128))
                                if c == 0 and g < 4:
                                    nc.sync.dma_start(
                                        wqb[:, 4096 * g:4096 * (g + 1)].rearrange(
                                            "p (k j) -> p k j", j=512),
                                        wqt[1024 * g:1024 * (g + 1), :].rearrange(
                                            "(k p) j -> p k j", p=128))
                                    nc.sync.dma_start(
                                        wkb[:, 1024 * g:1024 * (g + 1)].rearrange(
                                            "p (k j) -> p k j", j=128),
                                        wkt[1024 * g:1024 * (g + 1), :].rearrange(
                                            "(k p) j -> p k j", p=128))
                                    nc.sync.dma_start(
                                        wvb[:, 1024 * g:1024 * (g + 1)].rearrange(
                                            "p (k j) -> p k j", j=128),
                                        wvt[1024 * g:1024 * (g + 1), :].rearrange(
                                            "(k p) j -> p k j", p=128))
                                if c == 0 and g == 2:
                                    nc.sync.dma_start(cs_t[:, 0:SC], ctab[:, 0:SC])
                                    nc.sync.dma_start(sn_t[:, 0:SC], stab[:, 0:SC])
                                    nc.sync.dma_start(mc_t[:], mcaus[:])
                                    nc.sync.dma_start(mw_t[:], mwin[:])
                                if c == 0 and g >= 4:
                                    # wob halves ride the c0 tail (the weight
                                    # stream has drained by k=16)
                                    wh = 2 * (g - 4)
                                    for j in (wh, wh + 1):
                                        nc.sync.dma_start(
                                            wob[:, 2048 * j:2048 * (j + 1)],
                                            wot[128 * (j // 2):128 * (j // 2 + 1),
                                                4096 * (j % 2) // 2:
                                                4096 * (j % 2) // 2 + 2048])
                                if c == 1 and half == 0 and g == 0:
                                    nc.sync.dma_start(cs_t[:, SC:], ctab[:, SC:])
                                    nc.sync.dma_start(sn_t[:, SC:], stab[:, SC:])
                            hsl = hb[:, w * (k % 4):w * (k % 4 + 1)]
                            first, last = k == 0, k == KT - 1
                            for h in range(NQH):
                                nc.tensor.matmul(
                                    ps6[h][:, co:co + w],
                                    wqb[:, 512 * k + 128 * h:512 * k + 128 * (h + 1)],
                                    hsl, start=first, stop=last, skip_group_check=True)
                            nc.tensor.matmul(ps6[4][:, co:co + w],
                                             wkb[:, 128 * k:128 * (k + 1)], hsl,
                                             start=first, stop=last, skip_group_check=True)
                            nc.tensor.matmul(ps6[5][:, co:co + w],
                                             wvb[:, 128 * k:128 * (k + 1)], hsl,
                                             start=first, stop=last, skip_group_check=True)
                            if it >= 8 and wo_state["m"] < 24:
                                emit_wo_group()
                            it += 1
                        if half == halves - 1:
                            # their ACT copies precede the rope pre-copies in
                            # ACT's in-order queue, keeping the po-bank ring
                            # turning while DVE ropes
                            emit_wo_group()
                            emit_wo_group()
                        for r0 in range(0, w, 256):
                            # q1 before K on DVE: the attn score pool reuses
                            # the q0/q1 PSUM banks, so drain those first (q0
                            # frees via its ACT pre-copies)
                            _rope(qtb[0][:, SC * c + co + r0:SC * c + co + r0 + 256],
                                  ps6[0][:, co + r0:co + r0 + 256], c, co + r0, 256,
                                  eng="gp")
                            _rope(qtb[1][:, SC * c + co + r0:SC * c + co + r0 + 256],
                                  ps6[1][:, co + r0:co + r0 + 256], c, co + r0, 256,
                                  eng="dve")
                            _rope(ktb[:, SC * c + co + r0:SC * c + co + r0 + 256],
                                  ps6[4][:, co + r0:co + r0 + 256], c, co + r0, 256,
                                  eng="dve")
                        _rope(qtb[2][:, SC * c + co:SC * c + co + w],
                              ps6[2][:, co:co + w], c, co, w, eng="dve")
                        _rope(qtb[3][:, SC * c + co:SC * c + co + w],
                              ps6[3][:, co:co + w], c, co, w, eng="dve")
                        nc.scalar.copy(vtb[:, SC * c + co:SC * c + co + w],
                                       ps6[5][:, co:co + w])
                        for b2 in range(w // 128):
                            bo = 128 * ((SC // 128) * c + (w // 128) * half + b2)
                            nc.scalar.dma_start_transpose(
                                vnat[:, bo:bo + 128],
                                vtb[:, SC * c + co + 128 * b2:SC * c + co + 128 * (b2 + 1)])

            def attn_stage(c):
                # block-sparse attention for chunk c (past K/V only: sliding
                # window); leftover Wo groups of chunk c-1 fill the rope tail.
                with tc.tile_pool(name="scps", bufs=3, space="PSUM") as psc, \
                     tc.tile_pool(name="pvps", bufs=2, space="PSUM") as ppv, \
                     tc.tile_pool(name="denps", bufs=1, space="PSUM") as pdn:
                    kbs = list(range(max(0, 4 * c - 8), 4 * c + 4))
                    first_kb, last_kb = kbs[0], kbs[-1]
                    while emit_wo_group():
                        pass
                    atbs = []
                    for h0 in range(0, NQH, 2):
                        # process a PAIR of heads per key-block sweep: two
                        # independent score/exp chains per step keep ACT fed.
                        pvs = [ppv.tile([128, 512], F32, tag="pv", name="pv")
                               for _ in range(2)]
                        # one PSUM bank holds both heads' denominator rows
                        # (matmul out base partition must be 0/32/64)
                        pdnt = pdn.tile([33, 512], F32, tag="den", name="den")
                        dens = [pdnt[0:1, :], pdnt[32:33, :]]

                        def emit_pv(kb, pts):
                            # accumulate P@V and row-sums over exact visible slices.
                            sl = kb - 4 * c + 8
                            lo, hi = _slot_region(sl)
                            vsl = vnat[:, 128 * kb:128 * (kb + 1)]
                            for i in range(2):
                                nc.tensor.matmul(pvs[i][:, lo:hi], vsl, pts[i][:, lo:hi],
                                                 start=(kb == first_kb), stop=(kb == last_kb),
                                                 skip_group_check=True)
                                nc.tensor.matmul(dens[i][:, lo:hi], ones_t[:], pts[i][:, lo:hi],
                                                 start=(kb == first_kb), stop=(kb == last_kb),
                                                 skip_group_check=True)

                        pending = []
                        for kb in kbs:
                            sl = kb - 4 * c + 8
                            lo, hi = _slot_region(sl)
                            pts = []
                            for i in range(2):
                                sc = psc.tile([128, 512], F32, tag="sc", name="sc")
                                nc.tensor.matmul(sc[:, lo:hi], ktb[:, 128 * kb:128 * (kb + 1)],
                                                 qtb[h0 + i][:, SC * c + lo:SC * c + hi],
                                                 start=True, stop=True)
                                pt = ppt.tile([128, 512], BF16, tag="pt", name="pt")
                                nc.scalar.activation(pt[:, lo:hi], sc[:, lo:hi], EXP, scale=SCALE)
                                if sl <= 3:
                                    mofs = 128 * sl
                                    nc.vector.tensor_tensor(pt[:, mofs:mofs + 128],
                                                            pt[:, mofs:mofs + 128], mw_t[:], MULT)
                                elif sl >= 8:
                                    mofs = 128 * (sl - 8)
                                    nc.vector.tensor_tensor(pt[:, mofs:mofs + 128],
                                                            pt[:, mofs:mofs + 128], mc_t[:], MULT)
                                pts.append(pt)
                            pending.append((kb, pts))
                            if len(pending) > 2:
                                emit_pv(*pending.pop(0))
                        for pv_item in pending:
                            emit_pv(*pv_item)
                        for i in range(2):
                            # stage P@V out of PSUM via ACT so the bank frees
                            # immediately; normalize from SBUF off the
                            # critical path
                            pvu = pat.tile([128, 512], BF16, tag="pvu",
                                           name="pvu", bufs=2)
                            nc.scalar.copy(pvu[:], pvs[i][:])
                            dre = pmisc.tile([1, 512], BF16, tag="denr", name="denr")
                            with nc.allow_low_precision(reason="softmax denom to bf16"):
                                nc.vector.reciprocal(dre[:], dens[i])
                            dbc = pmisc.tile([128, 512], BF16, tag="denb", name="denb")
                            nc.gpsimd.partition_broadcast(dbc[:], dre[:])
                            at = pat.tile([128, 512], BF16, tag="atb", name="atb")
                            nc.vector.tensor_tensor(at[:], pvu[:], dbc[:], MULT)
                            atbs.append(at)
                return atbs

            for _it in range(niter):
                for c in range(NCH):
                    proj_stage(c)
                    atbs = attn_stage(c)
                    wo_state.update(c=c, atbs=atbs, m=0, obig=None)
            # the last chunk's Wo groups of each iteration ride the next
            # iteration's proj(0)/attn(0); only the final one drains here
            while emit_wo_group():
                pass


_NC_CACHE = {}


def _build(niter=1, fused=True):
    import os
    fused = os.environ.get("KERNEL_FUSED", "1" if fused else "0") == "1"
    key = (niter, fused)
    if key in _NC_CACHE:
        return _NC_CACHE[key]
    nc = bacc.Bacc("TRN2", target_bir_lowering=False, debug=False,
                   enable_asserts=True, num_devices=NCORES)
    dr = {}

    def din(name, shape, dt=F32):
        dr[name] = nc.dram_tensor(name, shape, dt, kind="ExternalInput").ap()

    din("ht", [HID, S], BF16)
    din("wqt", [HID, NQH * D], BF16)
    din("wkt", [HID, D], BF16)
    din("wvt", [HID, D], BF16)
    din("wot", [NQH * D, HID], BF16)
    din("ctab", [128, S])
    din("stab", [128, S])
    din("mcaus", [128, 128], BF16)
    din("mwin", [128, 128], BF16)
    out = nc.dram_tensor("out", [S, HID], BF16, kind="ExternalOutput").ap()

    with tile.TileContext(nc) as tc:
        _program(tc, dr, out, niter, fused)
    nc.compile()
    _NC_CACHE[key] = nc
    return nc


def make_in_maps(inputs):
    hs = np.asarray(inputs["hidden_states"], dtype=np.float32)
    Wq = np.asarray(inputs["Wq"], dtype=np.float32)
    Wk = np.asarray(inputs["Wk"], dtype=np.float32)
    Wv = np.asarray(inputs["Wv"], dtype=np.float32)
    Wo = np.asarray(inputs["Wo"], dtype=np.float32)
    pos = np.asarray(inputs["position_ids"]).reshape(-1)

    assert hs.shape == (1, S, HID), hs.shape
    H = hs[0]
    HT = np.ascontiguousarray(H.T)

    # RoPE tables in [d%64, s] layout (fp32, mirroring the reference math)
    inv = (1.0 / (ROPE_BASE ** (np.arange(0, D, 2, dtype=np.float32) / D))).astype(np.float32)
    ang = pos.astype(np.float32)[None, :] * inv[:, None]          # [64, S]
    cos64 = np.cos(ang).astype(np.float32)
    sin64 = np.sin(ang).astype(np.float32)
    ctab = np.concatenate([cos64, cos64], axis=0)                 # [128, S]
    stab = np.concatenate([-sin64, sin64], axis=0)                # sign-baked

    kk = np.arange(128)[:, None]
    qq = np.arange(128)[None, :]
    mcaus = (qq >= kk).astype(ml_dtypes.bfloat16)   # causal diag block, [k,q]
    mwin = (qq < kk).astype(ml_dtypes.bfloat16)     # window-edge block, [k,q]

    BF = ml_dtypes.bfloat16
    HTB = HT.astype(BF)
    in_maps = []
    for i in range(NCORES):
        in_maps.append({
            "ht": HTB,
            "wqt": np.ascontiguousarray(Wq[512 * i:512 * (i + 1), :].T).astype(BF),
            "wkt": np.ascontiguousarray(Wk[128 * i:128 * (i + 1), :].T).astype(BF),
            "wvt": np.ascontiguousarray(Wv[128 * i:128 * (i + 1), :].T).astype(BF),
            "wot": np.ascontiguousarray(Wo[:, 512 * i:512 * (i + 1)].T).astype(BF),
            "ctab": ctab,
            "stab": stab,
            "mcaus": mcaus,
            "mwin": mwin,
        })

    return in_maps


def kernel(**inputs):
    in_maps = make_in_maps(inputs)
    nc = _build()
    res = run_bass_kernel_spmd(nc, in_maps, core_ids=list(range(NCORES)))

    acc = np.zeros((S, HID), dtype=np.float32)
    for r in res.results:
        acc += r["out"].astype(np.float32)
    return acc.reshape(1, S, HID)

